# revision 1
# baseline (speedup 1.0000x reference)
"""Devign-GGNN Trainium2 kernel.

Full inputs in, full output out. Sharding: data-parallel over the B=32
graphs -> 4 graphs per NeuronCore on 8 cores. The gather/scatter message
passing is reformulated as dense per-(graph, etype) adjacency matmuls:

    a = sum_k A_k^T (h @ W_k) + bias_term,   A_k[s, d] = #edges(s->d, type k)

A (4 x 13 x 512 x 512 per core) is exact in fp8e4m3 (small integer counts)
and stays resident in SBUF; matmuls run bf16 x fp8 with fp32 PSUM
accumulation. Everything (GGNN steps, GRU, convs, readout) runs in one
fully-unrolled NEFF per core.

Layouts are transposed host-side so the hidden dim lives on SBUF
partitions: h^T is [256(=2x128 chunks), 2048 nodes] per core.
"""

import sys

if "/opt/trn_rl_repo" not in sys.path:
    sys.path.insert(0, "/opt/trn_rl_repo")

import numpy as np
import ml_dtypes

B, NPG, HID, NET, E, STEPS = 32, 512, 256, 13, 262144, 6
NCORES = 8
GPC = B // NCORES          # graphs per core = 4
NLOC = GPC * NPG           # local nodes = 2048
EPG = E // B               # edges per graph = 8192

_CACHE = {}


def _build_nc(steps=None, conv=None, skew=2, ell_delay=8, copy_mode="alt"):
    steps = STEPS if steps is None else steps
    conv = True if conv is None else conv
    import concourse.bass as bass  # noqa: F401
    import concourse.tile as tile
    from concourse import mybir, bacc
    from contextlib import ExitStack

    f32 = mybir.dt.float32
    bf16 = mybir.dt.bfloat16
    f8 = mybir.dt.float8e4
    AF = mybir.ActivationFunctionType
    X = mybir.AxisListType.X

    nc = bacc.Bacc(None, target_bir_lowering=False)

    xT_d = nc.dram_tensor("xT", [2, 128, NLOC], f32, kind="ExternalInput")
    A8_d = nc.dram_tensor("A8", [GPC, NET, 4, 128, NPG], f8, kind="ExternalInput")
    We_d = nc.dram_tensor("We", [2, 128, NET, HID], bf16, kind="ExternalInput")
    wih_d = nc.dram_tensor("wihT", [2, 128, 3 * HID], bf16, kind="ExternalInput")
    whh_d = nc.dram_tensor("whhT", [2, 128, 3 * HID], bf16, kind="ExternalInput")
    rzb_d = nc.dram_tensor("rzb", [128, 4], f32, kind="ExternalInput")
    ginb_d = nc.dram_tensor("ginb", [128, 2], f32, kind="ExternalInput")
    ghnb_d = nc.dram_tensor("ghnb", [128, 2], f32, kind="ExternalInput")
    bt_d = nc.dram_tensor("bt", [2, 128, NLOC], bf16, kind="ExternalInput")
    w1_d = nc.dram_tensor("w1D", [3, 128, 2, HID], f8, kind="ExternalInput")
    b1_d = nc.dram_tensor("b1", [128, 2], f32, kind="ExternalInput")
    w2_d = nc.dram_tensor("w2T", [2, 128, HID], bf16, kind="ExternalInput")
    b2_d = nc.dram_tensor("b2", [128, 2], f32, kind="ExternalInput")
    wc1_d = nc.dram_tensor("wc1D", [3, 128, 2, 2, 2 * HID], f8, kind="ExternalInput")
    bc1_d = nc.dram_tensor("bc1", [128, 4], f32, kind="ExternalInput")
    wc2_d = nc.dram_tensor("wc2T", [4, 128, 2 * HID], bf16, kind="ExternalInput")
    bc2_d = nc.dram_tensor("bc2", [128, 4], f32, kind="ExternalInput")
    wy_d = nc.dram_tensor("wy", [128, 2], bf16, kind="ExternalInput")
    wz_d = nc.dram_tensor("wz", [128, 4], bf16, kind="ExternalInput")
    byz_d = nc.dram_tensor("byz", [1, 2], f32, kind="ExternalInput")
    out_d = nc.dram_tensor("out", [1, GPC], f32, kind="ExternalOutput")

    with tile.TileContext(nc) as tc, ExitStack() as top:
        state = top.enter_context(tc.tile_pool(name="state", bufs=1))
        h_t = state.tile([128, 2, NLOC], f32)     # h (fp32 master state), hid-major
        hb_t = state.tile([128, 2, NLOC], bf16)   # bf16 shadow of h for matmuls


        cc = top.enter_context(tc.tile_pool(name="cc", bufs=1))
        w1_t = cc.tile([128, 3, 2, HID], f8)
        b1_t = cc.tile([128, 2], f32)
        w2_t = cc.tile([128, 2, HID], bf16)
        b2_t = cc.tile([128, 2], f32)
        wc1_t = cc.tile([128, 3, 2, 2, 2 * HID], f8)
        bc1_t = cc.tile([128, 4], f32)
        wc2_t = cc.tile([128, 4, 2 * HID], bf16)
        bc2_t = cc.tile([128, 4], f32)
        wy_t = cc.tile([128, 2], bf16)
        wz_t = cc.tile([128, 4], bf16)
        byz_t = cc.tile([1, 2], f32)
        hD_t = cc.tile([128, 2, NLOC], f8)
        xD_t = cc.tile([128, 2, NLOC], f8)

        # ---------------- GGNN: 6 message-passing + GRU steps ----------------
        with ExitStack() as gg:
            cg = gg.enter_context(tc.tile_pool(name="cg", bufs=1))
            We_t = cg.tile([128, 2, NET, HID], bf16)
            for k in range(2):
                nc.sync.dma_start(
                    We_t[:, :, k, :], We_d[:, :, k, :].rearrange("c p h -> p c h")
                )
            # graph 0 in half-graph chunks: the first stage-1 unit only
            # needs nodes 0:256, so compute starts after half the transfer
            for half in range(2):
                hsl = slice(half * 256, (half + 1) * 256)
                for kc in range(2):
                    nc.sync.dma_start(h_t[:, kc, hsl], xT_d[kc, :, hsl])
                    nc.scalar.copy(hb_t[:, kc, hsl], h_t[:, kc, hsl])
                    nc.vector.tensor_copy(xD_t[:, kc, hsl], h_t[:, kc, hsl])
            for k in range(2, NET):
                nc.sync.dma_start(
                    We_t[:, :, k, :], We_d[:, :, k, :].rearrange("c p h -> p c h")
                )
            bt_t = cg.tile([128, 2, NLOC], bf16)
            Ap = gg.enter_context(tc.tile_pool(name="Ap", bufs=1))
            A_t = Ap.tile([128, GPC, NET, 4, NPG], f8)
            for k in range(NET):
                nc.sync.dma_start(
                    A_t[:, 0, k, :, :], A8_d[0, k].rearrange("m p d -> p m d")
                )
            nc.sync.dma_start(
                bt_t[:, :, 0:NPG], bt_d[:, :, 0:NPG].rearrange("c p n -> p c n")
            )
            # remaining x graphs + small GRU consts
            for g in range(1, GPC):
                gsl = slice(g * NPG, (g + 1) * NPG)
                for kc in range(2):
                    nc.sync.dma_start(h_t[:, kc, gsl], xT_d[kc, :, gsl])
                    nc.scalar.copy(hb_t[:, kc, gsl], h_t[:, kc, gsl])
                    nc.vector.tensor_copy(xD_t[:, kc, gsl], h_t[:, kc, gsl])
            wih_t = cg.tile([128, 2, 3 * HID], bf16)
            nc.sync.dma_start(wih_t[:], wih_d.rearrange("c p m -> p c m"))
            whh_t = cg.tile([128, 2, 3 * HID], bf16)
            nc.sync.dma_start(whh_t[:], whh_d.rearrange("c p m -> p c m"))
            rzb_t = cg.tile([128, 4], f32)
            nc.sync.dma_start(rzb_t[:], rzb_d[:])
            ginb_t = cg.tile([128, 2], f32)
            nc.sync.dma_start(ginb_t[:], ginb_d[:])
            ghnb_t = cg.tile([128, 2], f32)
            nc.sync.dma_start(ghnb_t[:], ghnb_d[:])
            for g in range(1, GPC):
                gsl = slice(g * NPG, (g + 1) * NPG)
                for k in range(NET):
                    nc.sync.dma_start(
                        A_t[:, g, k, :, :], A8_d[g, k].rearrange("m p d -> p m d")
                    )
                nc.sync.dma_start(
                    bt_t[:, :, gsl], bt_d[:, :, gsl].rearrange("c p n -> p c n")
                )

            nc.sync.dma_start(w1_t[:], w1_d.rearrange("t p c o -> p t c o"))
            nc.sync.dma_start(b1_t[:], b1_d[:])
            nc.sync.dma_start(w2_t[:], w2_d.rearrange("c p o -> p c o"))
            nc.sync.dma_start(b2_t[:], b2_d[:])
            nc.sync.dma_start(wc1_t[:], wc1_d.rearrange("t p a b o -> p t a b o"))
            nc.sync.dma_start(bc1_t[:], bc1_d[:])
            nc.sync.dma_start(wc2_t[:], wc2_d.rearrange("c p o -> p c o"))
            nc.sync.dma_start(bc2_t[:], bc2_d[:])
            nc.sync.dma_start(wy_t[:], wy_d[:])
            nc.sync.dma_start(wz_t[:], wz_d[:])
            nc.sync.dma_start(byz_t[:], byz_d[:])


            ps_tn = top.enter_context(tc.tile_pool(name="ps_tn", bufs=3, space="PSUM"))
            ps_aT = top.enter_context(tc.tile_pool(name="ps_aT", bufs=1, space="PSUM"))
            ps_gru = top.enter_context(tc.tile_pool(name="ps_gru", bufs=2, space="PSUM"))
            tn_p = gg.enter_context(tc.tile_pool(name="tn", bufs=4))
            wk = gg.enter_context(tc.tile_pool(name="wk", bufs=2))
            wk1 = gg.enter_context(tc.tile_pool(name="wk1", bufs=1))

            # ---- global software pipeline over all (step, graph) blocks ----
            # Stage-2 DoubleRow matmuls for unit i are emitted SKEW units
            # later; the pipeline runs straight through block boundaries, so
            # the next block's stage-1 matmuls cover the previous block's
            # stage-2 tail + aT-evacuation + GRU-gate latency.
            SKEW = skew
            kgroups = [(2 * q, min(2, NET - 2 * q)) for q in range((NET + 1) // 2)]
            units = [(k0, nk, pi) for (k0, nk) in kgroups for pi in range(2)]
            NU = len(units)
            blocks = [(s, g) for s in range(steps) for g in range(GPC)]
            pend = []  # (tnD, s, g, k, pi, idx_in_block)
            aT_of = {}  # g -> live aT_ps tile

            def emit_s1_half(s, g, k0, nk, pi, j, tnD):
                w = nk * HID
                m = g * 4 + 2 * pi + j
                msl = slice(m * 128, (m + 1) * 128)
                tp = ps_tn.tile([128, 2 * HID], f32, name="tn_ps", tag="tn_ps")
                nc.tensor.matmul(
                    tp[:, :w], hb_t[:, 0, msl], We_t[:, 0, k0:k0 + nk, :],
                    start=True, stop=False,
                )
                nc.tensor.matmul(
                    tp[:, :w], hb_t[:, 1, msl], We_t[:, 1, k0:k0 + nk, :],
                    start=False, stop=True,
                )
                if j == 0:
                    nc.scalar.copy(tnD[:, j, :w], tp[:, :w])
                else:
                    nc.vector.tensor_copy(tnD[:, j, :w], tp[:, :w])

            def emit_s2_half(item, hc):
                tnD, s, g, k0, nk, pi, idx = item
                if idx == 0 and hc == 0:
                    aT_of[g] = ps_aT.tile([128, 2, NPG], f32, name="aT_ps", tag="aT_ps")
                aT_ps = aT_of[g]
                for ko in range(nk):
                    nc.tensor.matmul(
                        aT_ps[:, hc, :],
                        tnD[:, :, ko * HID + hc * 128:ko * HID + (hc + 1) * 128],
                        A_t[:, g, k0 + ko, 2 * pi:2 * pi + 2, :],
                        start=(idx == 0 and ko == 0),
                        stop=(idx == NU - 1 and ko == nk - 1),
                        perf_mode=mybir.MatmulPerfMode.DoubleRow,
                    )

            def emit_gru(s, g):
                gsl = slice(g * NPG, (g + 1) * NPG)
                aT_ps = aT_of.pop(g)
                # aggregated messages + per-node bias term
                aT_sb = wk.tile([128, 2, NPG], bf16, tag="aT")
                for hc in range(2):
                    nc.vector.tensor_add(
                        aT_sb[:, hc, :], aT_ps[:, hc, :], bt_t[:, hc, gsl]
                    )
                r_t = wk1.tile([128, 2, NPG], f32, tag="r")
                z_t = wk1.tile([128, 2, NPG], f32, tag="z")
                # whh-side matmuls first (depend only on hb, ready early);
                # the wih-side accumulates on top once aT_sb lands.
                gate_ps = {}

                def rz_whh(jc):
                    p_ = ps_gru.tile([128, NPG], f32, tag="gru")
                    gate_ps[jc] = p_
                    csl = slice(jc * 128, (jc + 1) * 128)
                    for kc in range(2):
                        nc.tensor.matmul(
                            p_[:], whh_t[:, kc, csl], hb_t[:, kc, gsl],
                            start=(kc == 0), stop=False,
                        )

                def rz_wih(jc):
                    p_ = gate_ps.pop(jc)
                    csl = slice(jc * 128, (jc + 1) * 128)
                    for kc in range(2):
                        nc.tensor.matmul(
                            p_[:], wih_t[:, kc, csl], aT_sb[:, kc, :],
                            start=False, stop=(kc == 1),
                        )
                    dst = r_t if jc < 2 else z_t
                    nc.scalar.activation(
                        dst[:, jc % 2, :], p_[:], AF.Sigmoid,
                        bias=rzb_t[:, jc:jc + 1],
                    )

                rz_whh(0)
                rz_whh(1)
                rz_wih(0)
                rz_whh(2)
                rz_wih(1)
                rz_whh(3)
                rz_wih(2)
                rz_wih(3)
                ginb = wk1.tile([128, 2, NPG], f32, tag="gin")
                hnb = wk1.tile([128, 2, NPG], f32, tag="hn")
                for jc in range(2):
                    csl = slice(512 + jc * 128, 512 + (jc + 1) * 128)
                    p2 = ps_gru.tile([128, NPG], f32, tag="gru")
                    for kc in range(2):
                        nc.tensor.matmul(
                            p2[:], whh_t[:, kc, csl], hb_t[:, kc, gsl],
                            start=(kc == 0), stop=(kc == 1),
                        )
                    nc.scalar.activation(
                        hnb[:, jc, :], p2[:], AF.Identity,
                        bias=ghnb_t[:, jc:jc + 1],
                    )
                    p_ = ps_gru.tile([128, NPG], f32, tag="gru")
                    for kc in range(2):
                        nc.tensor.matmul(
                            p_[:], wih_t[:, kc, csl], aT_sb[:, kc, :],
                            start=(kc == 0), stop=(kc == 1),
                        )
                    nc.scalar.activation(
                        ginb[:, jc, :], p_[:], AF.Identity,
                        bias=ginb_t[:, jc:jc + 1],
                    )

                # n = tanh(gin + r*hn); h' = n + z*(h - n)
                # r_t doubles as the n-gate scratch (r dead after first mul).
                # Deferred: emitted DELAY units later so these DVE ops queue
                # behind (not ahead of) the next block's tn copies.
                def ell():
                    tmp = r_t
                    for hc in range(2):
                        nc.vector.tensor_mul(tmp[:, hc, :], r_t[:, hc, :], hnb[:, hc, :])
                        nc.vector.tensor_add(tmp[:, hc, :], tmp[:, hc, :], ginb[:, hc, :])
                        nc.scalar.activation(tmp[:, hc, :], tmp[:, hc, :], AF.Tanh)
                        nc.vector.tensor_sub(hnb[:, hc, :], h_t[:, hc, gsl], tmp[:, hc, :])
                        nc.vector.tensor_mul(hnb[:, hc, :], hnb[:, hc, :], z_t[:, hc, :])
                        nc.vector.tensor_add(h_t[:, hc, gsl], tmp[:, hc, :], hnb[:, hc, :])
                        if s == steps - 1:
                            # final state: feed the conv head's fp8 shadow
                            nc.scalar.copy(hD_t[:, hc, gsl], h_t[:, hc, gsl])
                        else:
                            nc.scalar.copy(hb_t[:, hc, gsl], h_t[:, hc, gsl])
                return ell

            ELL_DELAY = ell_delay
            ell_q = []  # (due_tick, closure)
            tick = 0

            def pop_tail(item):
                _, ps, pg, _, _, _, pidx = item
                if pidx == NU - 1:
                    ell_q.append((tick + ELL_DELAY, emit_gru(ps, pg)))

            for (s, g) in blocks:
                for idx, (k0, nk, pi) in enumerate(units):
                    tick += 1
                    tnD = tn_p.tile([128, 2, 2 * HID], f8)
                    pend.append((tnD, s, g, k0, nk, pi, idx))
                    # interleave: each stage-2 half's ldweights hides under a
                    # 213ns stage-1 stream instead of a 107ns stage-2 one
                    emit_s1_half(s, g, k0, nk, pi, 0, tnD)
                    if len(pend) > SKEW:
                        emit_s2_half(pend[0], 0)
                    emit_s1_half(s, g, k0, nk, pi, 1, tnD)
                    if len(pend) > SKEW:
                        item = pend.pop(0)
                        emit_s2_half(item, 1)
                        pop_tail(item)
                    while ell_q and ell_q[0][0] <= tick:
                        ell_q.pop(0)[1]()
            while pend:
                item = pend.pop(0)
                emit_s2_half(item, 0)
                emit_s2_half(item, 1)
                pop_tail(item)
            while ell_q:
                ell_q.pop(0)[1]()

        # ---------------- conv head + readout ----------------
        with ExitStack() as cv:
          if conv:
              outp = cv.enter_context(tc.tile_pool(name="outp", bufs=1))
              out_sb = outp.tile([1, GPC], f32)
              cw = cv.enter_context(tc.tile_pool(name="cw", bufs=2))
              psc = ps_gru
              psr = ps_aT

              L1, L2, L3 = 510, 254, 127
              # Phase A: stage-1 convs (K=3) + maxpools for ALL graphs, so
              # phase B's matmuls always have cross-graph PE cover while the
              # DVE pools run.
              y1s, z1s = [], []
              for g in range(GPC):
                  gof = g * NPG
                  # conv1 (K=3, DoubleRow over the 2 ci chunks) + relu on h
                  y1p = cw.tile([128, 2, L1], f32, tag="y1p")
                  for co in range(2):
                      p_ = psc.tile([128, L1], f32, tag="gru")
                      for t in range(3):
                          nc.tensor.matmul(
                              p_[:],
                              w1_t[:, t, :, co * 128:(co + 1) * 128],
                              hD_t[:, :, gof + t:gof + t + L1],
                              start=(t == 0), stop=(t == 2),
                              perf_mode=mybir.MatmulPerfMode.DoubleRow,
                          )
                      nc.scalar.activation(
                          y1p[:, co, :], p_[:], AF.Relu, bias=b1_t[:, co:co + 1]
                      )
                  # cconv1 (K=3, C=512 over [h; x], DoubleRow pairs) + relu
                  z1p = cw.tile([128, 4, L1], f32, tag="z1p")
                  for co in range(4):
                      p_ = psc.tile([128, L1], f32, tag="gru")
                      idx = 0
                      for t in range(3):
                          for pr in range(2):
                              rhs = (hD_t if pr == 0 else xD_t)[
                                  :, :, gof + t:gof + t + L1
                              ]
                              nc.tensor.matmul(
                                  p_[:],
                                  wc1_t[:, t, pr, :, co * 128:(co + 1) * 128],
                                  rhs,
                                  start=(idx == 0), stop=(idx == 5),
                                  perf_mode=mybir.MatmulPerfMode.DoubleRow,
                              )
                              idx += 1
                      nc.scalar.activation(
                          z1p[:, co, :], p_[:], AF.Relu, bias=bc1_t[:, co:co + 1]
                      )
                  # maxpool k=3 s=2 -> 254
                  y1 = cw.tile([128, 2, L2], bf16, tag="y1", bufs=4)
                  tp = cw.tile([128, L2], f32, tag="tp")
                  for co in range(2):
                      e = y1p[:, co, :].rearrange("p (l s) -> p l s", s=2)
                      nc.vector.tensor_max(tp[:], e[:, :L2, 0], e[:, :L2, 1])
                      nc.vector.tensor_max(y1[:, co, :], tp[:], e[:, 1:L2 + 1, 0])
                  z1 = cw.tile([128, 4, L2], bf16, tag="z1", bufs=4)
                  for co in range(4):
                      e = z1p[:, co, :].rearrange("p (l s) -> p l s", s=2)
                      nc.vector.tensor_max(tp[:], e[:, :L2, 0], e[:, :L2, 1])
                      nc.vector.tensor_max(z1[:, co, :], tp[:], e[:, 1:L2 + 1, 0])
                  y1s.append(y1)
                  z1s.append(z1)

              # Phase B: K=1 convs + final pools + readout per graph.
              y2s, z2s = [], []
              for g in range(GPC):
                  y1, z1 = y1s[g], z1s[g]
                  # conv2 (K=1) + relu
                  y2p = cw.tile([128, 2, L2], f32, tag="y2p")
                  for co in range(2):
                      p_ = psc.tile([128, L2], f32, tag="gru")
                      for ci in range(2):
                          nc.tensor.matmul(
                              p_[:], w2_t[:, ci, co * 128:(co + 1) * 128], y1[:, ci, :],
                              start=(ci == 0), stop=(ci == 1),
                          )
                      nc.scalar.activation(
                          y2p[:, co, :], p_[:], AF.Relu, bias=b2_t[:, co:co + 1]
                      )
                  z2p = cw.tile([128, 4, L2], f32, tag="z2p")
                  for co in range(4):
                      p_ = psc.tile([128, L2], f32, tag="gru")
                      for ci in range(4):
                          nc.tensor.matmul(
                              p_[:], wc2_t[:, ci, co * 128:(co + 1) * 128], z1[:, ci, :],
                              start=(ci == 0), stop=(ci == 3),
                          )
                      nc.scalar.activation(
                          z2p[:, co, :], p_[:], AF.Relu, bias=bc2_t[:, co:co + 1]
                      )
                  # maxpool k=2 s=2 -> 127
                  y2 = cw.tile([128, 2, L3], bf16, tag="y2", bufs=4)
                  for co in range(2):
                      e = y2p[:, co, :].rearrange("p (l s) -> p l s", s=2)
                      nc.vector.tensor_max(y2[:, co, :], e[:, :, 0], e[:, :, 1])
                  z2 = cw.tile([128, 4, L3], bf16, tag="z2", bufs=4)
                  for co in range(4):
                      e = z2p[:, co, :].rearrange("p (l s) -> p l s", s=2)
                      nc.vector.tensor_max(z2[:, co, :], e[:, :, 0], e[:, :, 1])

                  y2s.append(y2)
                  z2s.append(z2)

              # Phase B2: readouts, covered by each other's matmuls
              for g in range(GPC):
                  y2, z2 = y2s[g], z2s[g]
                  # readout: sigmoid(mean((Y2 wy+by)*(Z2 wz+bz)))
                  zp = psr.tile([1, L3], f32, tag="rp")
                  for ci in range(4):
                      nc.tensor.matmul(
                          zp[:], wz_t[:, ci:ci + 1], z2[:, ci, :],
                          start=(ci == 0), stop=(ci == 3),
                      )
                  zb = cw.tile([1, L3], f32, tag="zb")
                  nc.vector.tensor_scalar_add(zb[:], zp[:], byz_t[:1, 1:2])
                  yp = psr.tile([1, L3], f32, tag="rp")
                  for hc in range(2):
                      nc.tensor.matmul(
                          yp[:], wy_t[:, hc:hc + 1], y2[:, hc, :],
                          start=(hc == 0), stop=(hc == 1),
                      )
                  # (yp+by)*zb with the row-sum fused via accum_out
                  yb = cw.tile([1, L3], f32, tag="yb")
                  sacc = cw.tile([1, 1], f32, tag="sacc")
                  nc.vector.scalar_tensor_tensor(
                      yb[:], yp[:], byz_t[:1, 0:1], zb[:],
                      op0=mybir.AluOpType.add, op1=mybir.AluOpType.mult,
                      accum_out=sacc[:],
                  )
                  nc.scalar.activation(
                      out_sb[:1, g:g + 1], sacc[:], AF.Sigmoid, scale=1.0 / L3
                  )
              nc.sync.dma_start(out_d[:], out_sb[:])

    nc.compile()
    return nc


def _host_prep(inputs):
    """Full inputs -> list of 8 per-core input dicts."""
    bf16 = ml_dtypes.bfloat16
    f8 = ml_dtypes.float8_e4m3

    x = np.asarray(inputs["x"], np.float32)
    src = np.asarray(inputs["src"], np.int32)
    dst = np.asarray(inputs["dst"], np.int32)
    et = np.asarray(inputs["etype"], np.int32)
    W_e = np.asarray(inputs["W_e"], np.float32)
    b_e = np.asarray(inputs["b_e"], np.float32)
    wih = np.asarray(inputs["gru_wih"], np.float32)
    whh = np.asarray(inputs["gru_whh"], np.float32)
    bih = np.asarray(inputs["gru_bih"], np.float32)
    bhh = np.asarray(inputs["gru_bhh"], np.float32)

    shared = {
        "We": np.ascontiguousarray(
            np.transpose(W_e.reshape(NET, 2, 128, HID), (1, 2, 0, 3))
        ).astype(bf16),
        "wihT": np.ascontiguousarray(wih.T.reshape(2, 128, 3 * HID)).astype(bf16),
        "whhT": np.ascontiguousarray(whh.T.reshape(2, 128, 3 * HID)).astype(bf16),
        "rzb": np.ascontiguousarray((bih + bhh)[: 2 * HID].reshape(4, 128).T).astype(
            np.float32
        ),
        "ginb": np.ascontiguousarray(bih[2 * HID:].reshape(2, 128).T).astype(np.float32),
        "ghnb": np.ascontiguousarray(bhh[2 * HID:].reshape(2, 128).T).astype(np.float32),
        "w1D": np.ascontiguousarray(
            np.transpose(
                np.transpose(np.asarray(inputs["conv1_w"], np.float32), (2, 1, 0))
                .reshape(3, 2, 128, HID), (0, 2, 1, 3)
            )
        ).astype(f8),
        "b1": np.ascontiguousarray(
            np.asarray(inputs["conv1_b"], np.float32).reshape(2, 128).T
        ),
        "w2T": np.ascontiguousarray(
            np.asarray(inputs["conv2_w"], np.float32)[:, :, 0].T.reshape(2, 128, HID)
        ).astype(bf16),
        "b2": np.ascontiguousarray(
            np.asarray(inputs["conv2_b"], np.float32).reshape(2, 128).T
        ),
        "wc1D": np.ascontiguousarray(
            np.transpose(
                np.transpose(np.asarray(inputs["cconv1_w"], np.float32), (2, 1, 0))
                .reshape(3, 2, 2, 128, 2 * HID), (0, 3, 1, 2, 4)
            )
        ).astype(f8),
        "bc1": np.ascontiguousarray(
            np.asarray(inputs["cconv1_b"], np.float32).reshape(4, 128).T
        ),
        "wc2T": np.ascontiguousarray(
            np.asarray(inputs["cconv2_w"], np.float32)[:, :, 0].T.reshape(
                4, 128, 2 * HID
            )
        ).astype(bf16),
        "bc2": np.ascontiguousarray(
            np.asarray(inputs["cconv2_b"], np.float32).reshape(4, 128).T
        ),
        "wy": np.ascontiguousarray(
            np.asarray(inputs["wy"], np.float32).reshape(2, 128).T
        ).astype(bf16),
        "wz": np.ascontiguousarray(
            np.asarray(inputs["wz"], np.float32).reshape(4, 128).T
        ).astype(bf16),
        "byz": np.array(
            [[float(np.asarray(inputs["by"]).reshape(-1)[0]),
              float(np.asarray(inputs["bz"]).reshape(-1)[0])]],
            np.float32,
        ),
    }

    in_maps = []
    for c in range(NCORES):
        n0 = c * NLOC
        esl = slice(c * GPC * EPG, (c + 1) * GPC * EPG)
        s_l = src[esl] - n0          # local node ids 0..2047
        d_l = dst[esl] - n0
        k_l = et[esl]
        g_l = s_l // NPG             # local graph 0..3 (edges stay in-graph)
        sg = s_l % NPG
        dg = d_l % NPG
        flat = ((g_l.astype(np.int64) * NET + k_l) * NPG + sg) * NPG + dg
        A = np.bincount(flat, minlength=GPC * NET * NPG * NPG).astype(f8)
        A8 = A.reshape(GPC, NET, 4, 128, NPG)

        bt = np.zeros((NLOC, HID), np.float32)
        np.add.at(bt, d_l, b_e[k_l])
        btT = np.ascontiguousarray(bt.T.reshape(2, 128, NLOC)).astype(bf16)

        xTc = np.ascontiguousarray(x[n0:n0 + NLOC].T.reshape(2, 128, NLOC))

        m = {"xT": xTc, "A8": A8, "bt": btT}
        m.update(shared)
        in_maps.append(m)
    return in_maps


def _get_nc():
    if "nc" not in _CACHE:
        _CACHE["nc"] = _build_nc()
    return _CACHE["nc"]


def run(inputs, trace=False):
    from concourse.bass_utils import run_bass_kernel_spmd

    nc = _get_nc()
    in_maps = _host_prep(inputs)
    res = run_bass_kernel_spmd(
        nc, in_maps, core_ids=list(range(NCORES)), trace=trace
    )
    out = np.concatenate(
        [np.asarray(res.results[c]["out"], np.float32).reshape(-1) for c in range(NCORES)]
    )
    return out, res


def kernel(**inputs):
    out, _ = run(inputs, trace=False)
    return out



# revision 55
# speedup vs baseline: 1.2323x; 1.2323x over previous
"""Devign-GGNN Trainium2 kernel.

Full inputs in, full output out. Sharding: data-parallel over the B=32
graphs -> 4 graphs per NeuronCore on 8 cores. The gather/scatter message
passing is reformulated as dense per-(graph, etype) adjacency matmuls:

    a = sum_k A_k^T (h @ W_k) + (indeg_k' b_k),  A_k[s, d] = #edges(s->d, k)

All three GGNN matmul families (h@W_e "stage 1", A^T "stage 2", GRU
gates) run as fp8e4m3 DoubleRow matmuls (contract 256/pass, 2 output
cols/cycle). W_e / GRU weights are scaled x8 host-side to sit in fp8's
normal range; the descale (x1/8) is folded into the PSUM-evacuation ops
(Act activation scale / DVE tensor_scalar) and gate activations.

h master state is bf16 (fp32 not needed; DVE gets 4x elementwise on
all-bf16 SBUF ops) with an fp8 shadow h8 feeding the matmuls. The
b_e aggregation is a per-block rank-13 matmul (indeg x b_e^T) that
seeds the stage-2 PSUM accumulator, replacing the host-side bincount.

Engine split per block: PE ~10.0us, Act (sigmoid/tanh + ~1/2 of the tn
PSUM->SBUF fp8 evacs) ~10us, DVE (stt/gates + other evacs) ~10us,
GpSimd/Pool (bf16 GRU state update; no PSUM port) ~7us.
"""

import sys

if "/opt/trn_rl_repo" not in sys.path:
    sys.path.insert(0, "/opt/trn_rl_repo")

import numpy as np
import ml_dtypes

B, NPG, HID, NET, E, STEPS = 32, 512, 256, 13, 262144, 6
NCORES = 8
GPC = B // NCORES          # graphs per core = 4
NLOC = GPC * NPG           # local nodes = 2048
EPG = E // B               # edges per graph = 8192

_CACHE = {}


def _build_nc(steps=None, conv=None, skew=6, gdl=(2, 3, 5, 6, 9),
              evac_pat=None, wk_bufs=3, merge_evac=True, tn_bufs=None,
              psum_mix=False, pool_cast=True):
    steps = STEPS if steps is None else steps
    conv = True if conv is None else conv
    import concourse.bass as bass  # noqa: F401
    import concourse.tile as tile
    from concourse import mybir, bacc
    from contextlib import ExitStack

    f32 = mybir.dt.float32
    bf16 = mybir.dt.bfloat16
    f8 = mybir.dt.float8e4
    AF = mybir.ActivationFunctionType
    ALU = mybir.AluOpType
    DR = mybir.MatmulPerfMode.DoubleRow

    nc = bacc.Bacc(None, target_bir_lowering=False)

    xT_d = nc.dram_tensor("xT", [2, 128, NLOC], bf16, kind="ExternalInput")
    xT8_d = nc.dram_tensor("xT8", [2, 128, NLOC], f8, kind="ExternalInput")
    A8_d = nc.dram_tensor("A8", [GPC, NET, 4, 128, NPG], f8, kind="ExternalInput")
    We_d = nc.dram_tensor("We8", [2, 128, NET * HID], f8, kind="ExternalInput")
    wih_d = nc.dram_tensor("wih8", [2, 128, 3 * HID], f8, kind="ExternalInput")
    whh_d = nc.dram_tensor("whh8", [2, 128, 3 * HID], f8, kind="ExternalInput")
    rzb_d = nc.dram_tensor("rzb", [128, 4], f32, kind="ExternalInput")
    ginb_d = nc.dram_tensor("ginb", [128, 2], f32, kind="ExternalInput")
    ghnb_d = nc.dram_tensor("ghnb8", [128, 2], f32, kind="ExternalInput")
    beT_d = nc.dram_tensor("beT", [NET, HID], bf16, kind="ExternalInput")
    indeg_d = nc.dram_tensor("indeg", [GPC, NET, NPG], bf16, kind="ExternalInput")
    w1_d = nc.dram_tensor("w1D", [3, 128, 2, HID], f8, kind="ExternalInput")
    b1_d = nc.dram_tensor("b1", [128, 2], f32, kind="ExternalInput")
    w2_d = nc.dram_tensor("w2T", [2, 128, HID], bf16, kind="ExternalInput")
    b2_d = nc.dram_tensor("b2", [128, 2], f32, kind="ExternalInput")
    wc1_d = nc.dram_tensor("wc1D", [3, 128, 2, 2, 2 * HID], f8, kind="ExternalInput")
    bc1_d = nc.dram_tensor("bc1", [128, 4], f32, kind="ExternalInput")
    wc2_d = nc.dram_tensor("wc2T", [4, 128, 2 * HID], bf16, kind="ExternalInput")
    bc2_d = nc.dram_tensor("bc2", [128, 4], f32, kind="ExternalInput")
    wy_d = nc.dram_tensor("wy", [128, 2], bf16, kind="ExternalInput")
    wz_d = nc.dram_tensor("wz", [128, 4], bf16, kind="ExternalInput")
    byz_d = nc.dram_tensor("byz", [1, 2], f32, kind="ExternalInput")
    out_d = nc.dram_tensor("out", [1, GPC], f32, kind="ExternalOutput")
    if not conv:
        hdump_d = nc.dram_tensor("hdump", [128, 2, NLOC], bf16, kind="ExternalOutput")
        adump_d = nc.dram_tensor("adump", [128, 2, NLOC], f8, kind="ExternalOutput")

    # evac engine cycle: 'A' = Act, 'D' = DVE (tn evacs + aT evac share it)
    if evac_pat is None:
        evac_pat = "ADDADDADDADDA" if merge_evac else "ADADADADADADADADADADADADDD"
    if tn_bufs is None:
        tn_bufs = 2 if merge_evac else 4

    with tile.TileContext(nc) as tc, ExitStack() as top:
        state = top.enter_context(tc.tile_pool(name="state", bufs=1))
        h_t = state.tile([128, 2, NLOC], bf16)    # h master (bf16), hid-major
        h8_t = state.tile([128, 2, NLOC], f8)     # fp8 shadow of h for matmuls
        xD_t = state.tile([128, 2, NLOC], f8)     # fp8 x (conv head)
        adump_t = None
        if not conv:
            adump_t = state.tile([128, 2, NLOC], f8)

        cc = top.enter_context(tc.tile_pool(name="cc", bufs=1))
        w1_t = cc.tile([128, 3, 2, HID], f8)
        b1_t = cc.tile([128, 2], f32)
        w2_t = cc.tile([128, 2, HID], bf16)
        b2_t = cc.tile([128, 2], f32)
        wc1_t = cc.tile([128, 3, 2, 2, 2 * HID], f8)
        bc1_t = cc.tile([128, 4], f32)
        wc2_t = cc.tile([128, 4, 2 * HID], bf16)
        bc2_t = cc.tile([128, 4], f32)
        wy_t = cc.tile([128, 2], bf16)
        wz_t = cc.tile([128, 4], bf16)
        byz_t = cc.tile([1, 2], f32)

        # ---------------- GGNN: 6 message-passing + GRU steps ----------------
        with ExitStack() as gg:
            cg = gg.enter_context(tc.tile_pool(name="cg", bufs=1))
            We_t = cg.tile([128, 2, NET * HID], f8)
            # first kgroup's We so unit 0 can start early
            nc.sync.dma_start(
                We_t[:, :, 0:2 * HID],
                We_d[:, :, 0:2 * HID].rearrange("c p h -> p c h"),
            )
            # graph 0 x in half-graph chunks: first units only need nodes 0:256
            for half in range(2):
                hsl = slice(half * 256, (half + 1) * 256)
                nc.sync.dma_start(h8_t[:, :, hsl], xT8_d[:, :, hsl].rearrange("c p n -> p c n"))
                nc.sync.dma_start(h_t[:, :, hsl], xT_d[:, :, hsl].rearrange("c p n -> p c n"))
            nc.sync.dma_start(
                We_t[:, :, 2 * HID:],
                We_d[:, :, 2 * HID:].rearrange("c p h -> p c h"),
            )
            Ap = gg.enter_context(tc.tile_pool(name="Ap", bufs=1))
            A_t = Ap.tile([128, GPC, NET, 4, NPG], f8)
            for k in range(NET):
                nc.sync.dma_start(
                    A_t[:, 0, k, :, :], A8_d[0, k].rearrange("m p d -> p m d")
                )
            wih_t = cg.tile([128, 2, 3 * HID], f8)
            nc.sync.dma_start(wih_t[:], wih_d.rearrange("c p m -> p c m"))
            whh_t = cg.tile([128, 2, 3 * HID], f8)
            nc.sync.dma_start(whh_t[:], whh_d.rearrange("c p m -> p c m"))
            rzb_t = cg.tile([128, 4], f32)
            nc.sync.dma_start(rzb_t[:], rzb_d[:])
            ginb_t = cg.tile([128, 2], f32)
            nc.sync.dma_start(ginb_t[:], ginb_d[:])
            ghnb_t = cg.tile([128, 2], f32)
            nc.sync.dma_start(ghnb_t[:], ghnb_d[:])
            beT_t = cg.tile([NET, HID], bf16)
            nc.sync.dma_start(beT_t[:], beT_d[:])
            indeg_t = cg.tile([NET, GPC, NPG], bf16)
            nc.sync.dma_start(indeg_t[:], indeg_d.rearrange("g k d -> k g d"))
            # remaining graphs
            for g in range(1, GPC):
                gsl = slice(g * NPG, (g + 1) * NPG)
                nc.sync.dma_start(h8_t[:, :, gsl], xT8_d[:, :, gsl].rearrange("c p n -> p c n"))
                nc.sync.dma_start(h_t[:, :, gsl], xT_d[:, :, gsl].rearrange("c p n -> p c n"))
                for k in range(NET):
                    nc.sync.dma_start(
                        A_t[:, g, k, :, :], A8_d[g, k].rearrange("m p d -> p m d")
                    )
            nc.sync.dma_start(xD_t[:], xT8_d.rearrange("c p n -> p c n"))

            nc.sync.dma_start(w1_t[:], w1_d.rearrange("t p c o -> p t c o"))
            nc.sync.dma_start(b1_t[:], b1_d[:])
            nc.sync.dma_start(w2_t[:], w2_d.rearrange("c p o -> p c o"))
            nc.sync.dma_start(b2_t[:], b2_d[:])
            nc.sync.dma_start(wc1_t[:], wc1_d.rearrange("t p a b o -> p t a b o"))
            nc.sync.dma_start(bc1_t[:], bc1_d[:])
            nc.sync.dma_start(wc2_t[:], wc2_d.rearrange("c p o -> p c o"))
            nc.sync.dma_start(bc2_t[:], bc2_d[:])
            nc.sync.dma_start(wy_t[:], wy_d[:])
            nc.sync.dma_start(wz_t[:], wz_d[:])
            nc.sync.dma_start(byz_t[:], byz_d[:])

            ps_tn = gg.enter_context(tc.tile_pool(name="ps_tn", bufs=tn_bufs, space="PSUM"))
            ps_aT = gg.enter_context(tc.tile_pool(name="ps_aT", bufs=1, space="PSUM"))
            ps_gru = gg.enter_context(tc.tile_pool(name="ps_gru", bufs=2, space="PSUM"))
            tn_p = gg.enter_context(tc.tile_pool(name="tn", bufs=6))
            wk = gg.enter_context(tc.tile_pool(name="wk", bufs=wk_bufs))

            # ---- global software pipeline over all (step, graph) blocks ----
            # unit = (k0, nk, pi) for nk=2 kgroups; the k=12 unit covers all
            # 4 node chunks at once. Each unit: S1 DR matmuls into a 2-bank
            # PSUM tile + one ap-1024 evac (x1/8 descale) to fp8 SBUF; S2 is
            # 4 DR matmuls vs the resident adjacency, emitted SKEW units later.
            SKEW = skew
            units = [(2 * q, 2, pi) for q in range(6) for pi in range(2)] + [(12, 1, 0)]
            NU = len(units)
            blocks = [(s, g) for s in range(steps) for g in range(GPC)]
            pend = []   # (tnD, s, g, k0, nk, pi, idx)
            aT_of = {}
            defq = []   # (due_tick, seq, closure)
            seqn = [0]
            tick = 0

            def defer(dt, fn):
                seqn[0] += 1
                defq.append((tick + dt, seqn[0], fn))

            def drain():
                defq.sort(key=lambda x: (x[0], x[1]))
                while defq and defq[0][0] <= tick:
                    defq.pop(0)[2]()

            evac_i = [0]

            def evac_engine():
                e = evac_pat[evac_i[0] % len(evac_pat)]
                evac_i[0] += 1
                return e

            def evac(dst_ap, src_ap):
                if evac_engine() == "A":
                    nc.scalar.activation(dst_ap, src_ap, AF.Identity, scale=0.125)
                else:
                    nc.vector.tensor_scalar_mul(dst_ap, src_ap, 0.125)

            def emit_s1(s, g, k0, nk, pi):
                # S1 PSUM: either two 1-bank tiles with two parallel ap-512
                # evacs, or one 2-bank tile with a single ap-1024 evac. The
                # fp8 SBUF tile is contiguous either way (the S2 DoubleRow
                # pair-dim spans both node-halves).
                if nk == 2:
                    tnD = tn_p.tile([128, 2, 2, HID], f8)
                    if merge_evac:
                        tp = ps_tn.tile([128, 2, 2, HID], f32, tag="tn_ps")
                        for j in range(2):
                            m = g * 4 + 2 * pi + j
                            msl = slice(m * 128, (m + 1) * 128)
                            nc.tensor.matmul(
                                tp[:, j, :, :], h8_t[:, :, msl],
                                We_t[:, :, k0 * HID:(k0 + nk) * HID],
                                start=True, stop=True, perf_mode=DR,
                            )
                        evac(tnD[:], tp[:])
                    else:
                        for j in range(2):
                            tp = ps_tn.tile([128, 2, HID], f32, tag="tn_ps")
                            m = g * 4 + 2 * pi + j
                            msl = slice(m * 128, (m + 1) * 128)
                            nc.tensor.matmul(
                                tp[:], h8_t[:, :, msl],
                                We_t[:, :, k0 * HID:(k0 + nk) * HID],
                                start=True, stop=True, perf_mode=DR,
                            )
                            evac(tnD[:, j, :, :], tp[:])
                else:
                    tnD = tn_p.tile([128, 4, HID], f8)
                    if merge_evac:
                        tp = ps_tn.tile([128, 4, HID], f32, tag="tn_ps")
                        for j in range(4):
                            m = g * 4 + j
                            msl = slice(m * 128, (m + 1) * 128)
                            nc.tensor.matmul(
                                tp[:, j, :], h8_t[:, :, msl],
                                We_t[:, :, k0 * HID:(k0 + 1) * HID],
                                start=True, stop=True, perf_mode=DR,
                            )
                        evac(tnD[:], tp[:])
                    else:
                        for q in range(2):
                            tp = ps_tn.tile([128, 2, HID], f32, tag="tn_ps")
                            for j in range(2):
                                m = g * 4 + 2 * q + j
                                msl = slice(m * 128, (m + 1) * 128)
                                nc.tensor.matmul(
                                    tp[:, j, :], h8_t[:, :, msl],
                                    We_t[:, :, k0 * HID:(k0 + 1) * HID],
                                    start=True, stop=True, perf_mode=DR,
                                )
                            evac(tnD[:, 2 * q:2 * q + 2, :], tp[:])
                return tnD

            def emit_s2(item):
                tnD, s, g, k0, nk, pi, idx = item
                if idx == 0:
                    aT_of[g] = ps_aT.tile([128, 2, NPG], f32, name="aT_ps", tag="aT_ps")
                    # seed accumulator with the aggregated edge-bias:
                    # aT[hid, d] = sum_k b_e[k, hid] * indeg_k[d]
                    for hc in range(2):
                        nc.tensor.matmul(
                            aT_of[g][:, hc, :],
                            beT_t[:, hc * 128:(hc + 1) * 128],
                            indeg_t[:, g, :],
                            start=True, stop=False,
                        )
                aT_ps = aT_of[g]
                last = idx == NU - 1
                if nk == 2:
                    for hc in range(2):
                        for ko in range(2):
                            nc.tensor.matmul(
                                aT_ps[:, hc, :],
                                tnD[:, :, ko, hc * 128:(hc + 1) * 128],
                                A_t[:, g, k0 + ko, 2 * pi:2 * pi + 2, :],
                                start=False,
                                stop=(last and ko == 1),
                                perf_mode=DR,
                            )
                else:
                    for hc in range(2):
                        for qi in range(2):
                            nc.tensor.matmul(
                                aT_ps[:, hc, :],
                                tnD[:, 2 * qi:2 * qi + 2, hc * 128:(hc + 1) * 128],
                                A_t[:, g, k0, 2 * qi:2 * qi + 2, :],
                                start=False,
                                stop=(last and qi == 1),
                                perf_mode=DR,
                            )

            def emit_gru(s, g):
                gsl = slice(g * NPG, (g + 1) * NPG)
                aT_ps = aT_of.pop(g)
                aT8 = wk.tile([128, 2, NPG], f8, tag="aT8")
                r_t = wk.tile([128, 2, NPG], bf16, tag="r")
                z_t = wk.tile([128, 2, NPG], bf16, tag="z")
                n_t = wk.tile([128, 2, NPG], bf16, tag="n")
                d_t = r_t  # r is dead after part_n's stt; reuse as ell scratch

                # now: evacuate aT (fp8 for the DR wih matmuls), one half per
                # engine so both run in parallel
                nc.scalar.copy(aT8[:, 0, :], aT_ps[:, 0, :])
                nc.vector.tensor_copy(aT8[:, 1, :], aT_ps[:, 1, :])
                if adump_t is not None and s == steps - 1:
                    nc.gpsimd.tensor_copy(adump_t[:, :, gsl], aT8[:])

                def gate_pair(jp):
                    # jp=0: r gates (jc 0,1); jp=1: z gates (jc 2,3)
                    def fn():
                        dst = r_t if jp == 0 else z_t
                        for hc in range(2):
                            jc = 2 * jp + hc
                            csl = slice(jc * 128, (jc + 1) * 128)
                            p_ = ps_gru.tile([128, NPG], f32, tag="gru")
                            nc.tensor.matmul(
                                p_[:], whh_t[:, :, csl], h8_t[:, :, gsl],
                                start=True, stop=False, perf_mode=DR,
                            )
                            nc.tensor.matmul(
                                p_[:], wih_t[:, :, csl], aT8[:],
                                start=False, stop=True, perf_mode=DR,
                            )
                            nc.scalar.activation(
                                dst[:, hc, :], p_[:], AF.Sigmoid,
                                bias=rzb_t[:, jc:jc + 1], scale=0.125,
                            )
                    return fn

                def n_gate(hc):
                    def fn():
                        csl = slice(512 + hc * 128, 512 + (hc + 1) * 128)
                        p_ = ps_gru.tile([128, NPG], f32, tag="gru")
                        nc.tensor.matmul(
                            p_[:], whh_t[:, :, csl], h8_t[:, :, gsl],
                            start=True, stop=True, perf_mode=DR,
                        )
                        if psum_mix:
                            # p = 8*r*(gh_n + bhh_n)  (in-place on PSUM, DVE)
                            nc.vector.scalar_tensor_tensor(
                                p_[:], p_[:], ghnb_t[:, hc:hc + 1], r_t[:, hc, :],
                                op0=ALU.add, op1=ALU.mult,
                            )
                            # p += 8*gi_n  (accumulate on top of the DVE value)
                            nc.tensor.matmul(
                                p_[:], wih_t[:, :, csl], aT8[:],
                                start=False, stop=True, perf_mode=DR,
                                skip_group_check=True,
                            )
                            nc.scalar.activation(
                                n_t[:, hc, :], p_[:], AF.Tanh,
                                bias=ginb_t[:, hc:hc + 1], scale=0.125,
                            )
                        else:
                            # safe path: d = 8*r*(gh_n + bhh_n) to SBUF, then
                            # gi_n in its own PSUM group, tanh(gi+d) via add
                            nc.vector.scalar_tensor_tensor(
                                d_t[:, hc, :], p_[:], ghnb_t[:, hc:hc + 1],
                                r_t[:, hc, :], op0=ALU.add, op1=ALU.mult,
                            )
                            p2 = ps_gru.tile([128, NPG], f32, tag="gru")
                            nc.tensor.matmul(
                                p2[:], wih_t[:, :, csl], aT8[:],
                                start=True, stop=True, perf_mode=DR,
                            )
                            nc.vector.tensor_add(
                                n_t[:, hc, :], p2[:], d_t[:, hc, :]
                            )
                            nc.scalar.activation(
                                n_t[:, hc, :], n_t[:, hc, :], AF.Tanh,
                                bias=ginb_t[:, hc:hc + 1], scale=0.125,
                            )
                    return fn

                def ell():
                    # h' = n + z*(h - n); all-bf16 SBUF chain hits DVE's 4x
                    # mode. GpSimd only does the fp8 shadow copy (TensorTensor
                    # arith fails the Pool ISA check on this toolchain).
                    for hc in range(2):
                        nc.vector.tensor_sub(d_t[:, hc, :], h_t[:, hc, gsl], n_t[:, hc, :])
                        nc.vector.tensor_mul(d_t[:, hc, :], d_t[:, hc, :], z_t[:, hc, :])
                        nc.vector.tensor_add(h_t[:, hc, gsl], n_t[:, hc, :], d_t[:, hc, :])
                        cast_eng = nc.gpsimd if pool_cast else nc.vector
                        cast_eng.tensor_copy(h8_t[:, hc, gsl], h_t[:, hc, gsl])

                defer(gdl[0], gate_pair(0))
                defer(gdl[1], gate_pair(1))
                defer(gdl[2], n_gate(0))
                defer(gdl[3], n_gate(1))
                defer(gdl[4], ell)

            for (s, g) in blocks:
                for idx, (k0, nk, pi) in enumerate(units):
                    tick += 1
                    tnD = emit_s1(s, g, k0, nk, pi)
                    pend.append((tnD, s, g, k0, nk, pi, idx))
                    if len(pend) > SKEW:
                        item = pend.pop(0)
                        emit_s2(item)
                        if item[6] == NU - 1:
                            emit_gru(item[1], item[2])
                    drain()
            while pend:
                tick += 1
                item = pend.pop(0)
                emit_s2(item)
                if item[6] == NU - 1:
                    emit_gru(item[1], item[2])
                drain()
            while defq:
                tick += 1
                drain()

        # ---------------- conv head + readout ----------------
        with ExitStack() as cv:
          if not conv:
              nc.sync.dma_start(hdump_d[:], h_t[:])
              nc.sync.dma_start(adump_d[:], adump_t[:])
              op = cv.enter_context(tc.tile_pool(name="outp", bufs=1))
              o_sb = op.tile([1, GPC], f32)
              nc.gpsimd.memset(o_sb[:], 0.0)
              nc.sync.dma_start(out_d[:], o_sb[:])
          if conv:
              outp = cv.enter_context(tc.tile_pool(name="outp", bufs=1))
              out_sb = outp.tile([1, GPC], f32)
              cw = cv.enter_context(tc.tile_pool(name="cw", bufs=2))
              psc = cv.enter_context(tc.tile_pool(name="psc", bufs=3, space="PSUM"))
              psr = cv.enter_context(tc.tile_pool(name="psr", bufs=1, space="PSUM"))

              L1, L2, L3 = 510, 254, 127
              # Phase A: stage-1 convs (K=3) + maxpools for ALL graphs
              y1s, z1s = [], []
              for g in range(GPC):
                  gof = g * NPG
                  # conv1 (K=3, DR over ci) + relu on h (h8_t is final h fp8)
                  y1p = cw.tile([128, 2, L1], bf16, tag="y1p")
                  for co in range(2):
                      p_ = psc.tile([128, L1], f32, tag="cps")
                      for t in range(3):
                          nc.tensor.matmul(
                              p_[:],
                              w1_t[:, t, :, co * 128:(co + 1) * 128],
                              h8_t[:, :, gof + t:gof + t + L1],
                              start=(t == 0), stop=(t == 2),
                              perf_mode=DR,
                          )
                      nc.scalar.activation(
                          y1p[:, co, :], p_[:], AF.Relu, bias=b1_t[:, co:co + 1]
                      )
                  # cconv1 (K=3, C=512 over [h; x], DR pairs) + relu
                  z1p = cw.tile([128, 4, L1], bf16, tag="z1p")
                  for co in range(4):
                      p_ = psc.tile([128, L1], f32, tag="cps")
                      idx = 0
                      for t in range(3):
                          for pr in range(2):
                              rhs = (h8_t if pr == 0 else xD_t)[
                                  :, :, gof + t:gof + t + L1
                              ]
                              nc.tensor.matmul(
                                  p_[:],
                                  wc1_t[:, t, pr, :, co * 128:(co + 1) * 128],
                                  rhs,
                                  start=(idx == 0), stop=(idx == 5),
                                  perf_mode=DR,
                              )
                              idx += 1
                      # relu on DVE: (psum + bias) max 0
                      nc.vector.tensor_scalar(
                          z1p[:, co, :], p_[:], bc1_t[:, co:co + 1], 0.0,
                          op0=ALU.add, op1=ALU.max,
                      )
                  # maxpool k=3 s=2 -> 254 (bf16, feeds bf16 K=1 convs)
                  y1 = cw.tile([128, 2, L2], bf16, tag="y1", bufs=4)
                  for co in range(2):
                      tp = cw.tile([128, L2], bf16, tag="tp", bufs=4)
                      e = y1p[:, co, :].rearrange("p (l s) -> p l s", s=2)
                      nc.vector.tensor_max(tp[:], e[:, :L2, 0], e[:, :L2, 1])
                      nc.vector.tensor_max(y1[:, co, :], tp[:], e[:, 1:L2 + 1, 0])
                  z1 = cw.tile([128, 4, L2], bf16, tag="z1", bufs=4)
                  for co in range(4):
                      tp = cw.tile([128, L2], bf16, tag="tp2", bufs=4)
                      e = z1p[:, co, :].rearrange("p (l s) -> p l s", s=2)
                      nc.vector.tensor_max(tp[:], e[:, :L2, 0], e[:, :L2, 1])
                      nc.vector.tensor_max(z1[:, co, :], tp[:], e[:, 1:L2 + 1, 0])
                  y1s.append(y1)
                  z1s.append(z1)

              # Phase B: K=1 convs (fp8 DR, weights x8 -> relu descale) + pools
              y2s, z2s = [], []
              for g in range(GPC):
                  y1, z1 = y1s[g], z1s[g]
                  y2p = cw.tile([128, 2, L2], bf16, tag="y2p")
                  for co in range(2):
                      p_ = psc.tile([128, L2], f32, tag="cps")
                      for ci in range(2):
                          nc.tensor.matmul(
                              p_[:], w2_t[:, ci, co * 128:(co + 1) * 128], y1[:, ci, :],
                              start=(ci == 0), stop=(ci == 1),
                          )
                      nc.scalar.activation(
                          y2p[:, co, :], p_[:], AF.Relu, bias=b2_t[:, co:co + 1]
                      )
                  z2p = cw.tile([128, 4, L2], bf16, tag="z2p")
                  for co in range(4):
                      p_ = psc.tile([128, L2], f32, tag="cps")
                      for ci in range(4):
                          nc.tensor.matmul(
                              p_[:], wc2_t[:, ci, co * 128:(co + 1) * 128], z1[:, ci, :],
                              start=(ci == 0), stop=(ci == 3),
                          )
                      nc.scalar.activation(
                          z2p[:, co, :], p_[:], AF.Relu, bias=bc2_t[:, co:co + 1]
                      )
                  # maxpool k=2 s=2 -> 127 (bf16, DVE 4x)
                  y2 = cw.tile([128, 2, L3], bf16, tag="y2", bufs=4)
                  for co in range(2):
                      e = y2p[:, co, :].rearrange("p (l s) -> p l s", s=2)
                      nc.vector.tensor_max(y2[:, co, :], e[:, :, 0], e[:, :, 1])
                  z2 = cw.tile([128, 4, L3], bf16, tag="z2", bufs=4)
                  for co in range(4):
                      e = z2p[:, co, :].rearrange("p (l s) -> p l s", s=2)
                      nc.vector.tensor_max(z2[:, co, :], e[:, :, 0], e[:, :, 1])

                  y2s.append(y2)
                  z2s.append(z2)

              # Phase B2: readouts, covered by each other's matmuls
              for g in range(GPC):
                  y2, z2 = y2s[g], z2s[g]
                  zp = psr.tile([1, L3], f32, tag="rp")
                  for ci in range(4):
                      nc.tensor.matmul(
                          zp[:], wz_t[:, ci:ci + 1], z2[:, ci, :],
                          start=(ci == 0), stop=(ci == 3),
                      )
                  zb = cw.tile([1, L3], f32, tag="zb")
                  nc.vector.tensor_scalar_add(zb[:], zp[:], byz_t[:1, 1:2])
                  yp = psr.tile([1, L3], f32, tag="rp")
                  for hc in range(2):
                      nc.tensor.matmul(
                          yp[:], wy_t[:, hc:hc + 1], y2[:, hc, :],
                          start=(hc == 0), stop=(hc == 1),
                      )
                  yb = cw.tile([1, L3], f32, tag="yb")
                  sacc = cw.tile([1, 1], f32, tag="sacc")
                  nc.vector.scalar_tensor_tensor(
                      yb[:], yp[:], byz_t[:1, 0:1], zb[:],
                      op0=ALU.add, op1=ALU.mult,
                      accum_out=sacc[:],
                  )
                  nc.scalar.activation(
                      out_sb[:1, g:g + 1], sacc[:], AF.Sigmoid, scale=1.0 / L3
                  )
              nc.sync.dma_start(out_d[:], out_sb[:])

    nc.compile()
    return nc


def _host_prep(inputs):
    """Full inputs -> list of 8 per-core input dicts."""
    bf16 = ml_dtypes.bfloat16
    f8 = ml_dtypes.float8_e4m3

    x = np.asarray(inputs["x"], np.float32)
    src = np.asarray(inputs["src"], np.int32)
    dst = np.asarray(inputs["dst"], np.int32)
    et = np.asarray(inputs["etype"], np.int32)
    W_e = np.asarray(inputs["W_e"], np.float32)
    b_e = np.asarray(inputs["b_e"], np.float32)
    wih = np.asarray(inputs["gru_wih"], np.float32)
    whh = np.asarray(inputs["gru_whh"], np.float32)
    bih = np.asarray(inputs["gru_bih"], np.float32)
    bhh = np.asarray(inputs["gru_bhh"], np.float32)

    def wT8(w):  # [out, in] -> [2, 128, out] fp8, x8 (chunk-major dram layout)
        return np.ascontiguousarray(
            (8.0 * w).T.reshape(2, 128, w.shape[0])
        ).astype(f8)

    shared = {
        "We8": np.ascontiguousarray(
            (8.0 * W_e).reshape(NET, 2, 128, HID).transpose(1, 2, 0, 3)
            .reshape(2, 128, NET * HID)
        ).astype(f8),
        "wih8": wT8(wih),
        "whh8": wT8(whh),
        "rzb": np.ascontiguousarray((bih + bhh)[: 2 * HID].reshape(4, 128).T).astype(
            np.float32
        ),
        "ginb": np.ascontiguousarray(bih[2 * HID:].reshape(2, 128).T).astype(np.float32),
        "ghnb8": np.ascontiguousarray(8.0 * bhh[2 * HID:].reshape(2, 128).T).astype(
            np.float32
        ),
        "beT": np.ascontiguousarray(b_e).astype(bf16),
        "w1D": np.ascontiguousarray(
            np.transpose(
                np.transpose(np.asarray(inputs["conv1_w"], np.float32), (2, 1, 0))
                .reshape(3, 2, 128, HID), (0, 2, 1, 3)
            )
        ).astype(f8),
        "b1": np.ascontiguousarray(
            np.asarray(inputs["conv1_b"], np.float32).reshape(2, 128).T
        ),
        "w2T": np.ascontiguousarray(
            np.asarray(inputs["conv2_w"], np.float32)[:, :, 0].T.reshape(2, 128, HID)
        ).astype(bf16),
        "b2": np.ascontiguousarray(
            np.asarray(inputs["conv2_b"], np.float32).reshape(2, 128).T
        ),
        "wc1D": np.ascontiguousarray(
            np.transpose(
                np.transpose(np.asarray(inputs["cconv1_w"], np.float32), (2, 1, 0))
                .reshape(3, 2, 2, 128, 2 * HID), (0, 3, 1, 2, 4)
            )
        ).astype(f8),
        "bc1": np.ascontiguousarray(
            np.asarray(inputs["cconv1_b"], np.float32).reshape(4, 128).T
        ),
        "wc2T": np.ascontiguousarray(
            np.asarray(inputs["cconv2_w"], np.float32)[:, :, 0].T.reshape(
                4, 128, 2 * HID
            )
        ).astype(bf16),
        "bc2": np.ascontiguousarray(
            np.asarray(inputs["cconv2_b"], np.float32).reshape(4, 128).T
        ),
        "wy": np.ascontiguousarray(
            np.asarray(inputs["wy"], np.float32).reshape(2, 128).T
        ).astype(bf16),
        "wz": np.ascontiguousarray(
            np.asarray(inputs["wz"], np.float32).reshape(4, 128).T
        ).astype(bf16),
        "byz": np.array(
            [[float(np.asarray(inputs["by"]).reshape(-1)[0]),
              float(np.asarray(inputs["bz"]).reshape(-1)[0])]],
            np.float32,
        ),
    }

    in_maps = []
    for c in range(NCORES):
        n0 = c * NLOC
        esl = slice(c * GPC * EPG, (c + 1) * GPC * EPG)
        s_l = src[esl] - n0          # local node ids 0..2047
        d_l = dst[esl] - n0
        k_l = et[esl]
        g_l = s_l // NPG             # local graph 0..3 (edges stay in-graph)
        sg = s_l % NPG
        dg = d_l % NPG
        flat = ((g_l.astype(np.int64) * NET + k_l) * NPG + sg) * NPG + dg
        A = np.bincount(flat, minlength=GPC * NET * NPG * NPG).astype(f8)
        A8 = A.reshape(GPC, NET, 4, 128, NPG)

        dflat = (g_l.astype(np.int64) * NET + k_l) * NPG + dg
        indeg = np.bincount(dflat, minlength=GPC * NET * NPG).astype(bf16)

        xTc = np.ascontiguousarray(x[n0:n0 + NLOC].T.reshape(2, 128, NLOC))

        m = {
            "xT": xTc.astype(bf16),
            "xT8": xTc.astype(f8),
            "A8": A8,
            "indeg": indeg.reshape(GPC, NET, NPG),
        }
        m.update(shared)
        in_maps.append(m)
    return in_maps


def _get_nc():
    if "nc" not in _CACHE:
        _CACHE["nc"] = _build_nc()
    return _CACHE["nc"]


def run(inputs, trace=False):
    from concourse.bass_utils import run_bass_kernel_spmd

    nc = _get_nc()
    in_maps = _host_prep(inputs)
    res = run_bass_kernel_spmd(
        nc, in_maps, core_ids=list(range(NCORES)), trace=trace
    )
    out = np.concatenate(
        [np.asarray(res.results[c]["out"], np.float32).reshape(-1) for c in range(NCORES)]
    )
    return out, res


def kernel(**inputs):
    out, _ = run(inputs, trace=False)
    return out


# revision 75
# speedup vs baseline: 1.5883x; 1.2889x over previous
"""Devign-GGNN Trainium2 kernel.

Full inputs in, full output out. Sharding: data-parallel over the B=32
graphs -> 4 graphs per NeuronCore on 8 cores. The gather/scatter message
passing is reformulated as dense per-(graph, etype) adjacency matmuls:

    a = sum_k A_k^T (h @ W_k) + (indeg_k' b_k),  A_k[s, d] = #edges(s->d, k)

All three GGNN matmul families (h@W_e "stage 1", A^T "stage 2", GRU
gates) run as fp8e4m3 DoubleRow matmuls (contract 256/pass, 2 output
cols/cycle). W_e / GRU weights are scaled x8 host-side to sit in fp8's
normal range; the descale (x1/8) is folded into the PSUM-evacuation ops
(Act activation scale / DVE tensor_scalar) and the gate activations.

h master state is bf16 (DVE gets its 4x elementwise mode on all-bf16
SBUF ops) with an fp8 shadow h8 feeding the matmuls. The b_e
aggregation is a per-block rank-13 matmul (indeg x b_e^T) seeding the
stage-2 PSUM accumulator, replacing the host-side bincount. The conv
head keeps baseline numerics (fp8 K=3 convs, bf16 K=1 convs) - head
precision feeds the readout directly, so it gets the extra bits.

Engine split per GGNN block: PE ~10us of DR matmuls; Act (sigmoids,
tanh, ~half the tn PSUM->SBUF fp8 evacuations) and DVE (stt gates, GRU
state update, remaining evacuations) ~11us each; GpSimd only holds the
h->fp8 shadow copies (no PSUM port, and TensorTensor fails its ISA
check on this toolchain).
"""

import sys

if "/opt/trn_rl_repo" not in sys.path:
    sys.path.insert(0, "/opt/trn_rl_repo")

import numpy as np
import ml_dtypes

B, NPG, HID, NET, E, STEPS = 32, 512, 256, 13, 262144, 6
NCORES = 8
GPC = B // NCORES          # graphs per core = 4
NLOC = GPC * NPG           # local nodes = 2048
EPG = E // B               # edges per graph = 8192

_CACHE = {}


def _build_nc(steps=None, conv=None, skew=18, gdl=(3, 4, 6, 7, 10),
              evac_pat=None, wk_bufs=3, merge_evac=True, tn_bufs=None,
              at_halves="AA", tnp_bufs=14):
    steps = STEPS if steps is None else steps
    conv = True if conv is None else conv
    import concourse.bass as bass  # noqa: F401
    import concourse.tile as tile
    from concourse import mybir, bacc
    from contextlib import ExitStack

    f32 = mybir.dt.float32
    bf16 = mybir.dt.bfloat16
    f8 = mybir.dt.float8e4
    AF = mybir.ActivationFunctionType
    ALU = mybir.AluOpType
    DR = mybir.MatmulPerfMode.DoubleRow

    nc = bacc.Bacc(None, target_bir_lowering=False)

    xT_d = nc.dram_tensor("xT", [2, 128, NLOC], bf16, kind="ExternalInput")
    xT8_d = nc.dram_tensor("xT8", [2, 128, NLOC], f8, kind="ExternalInput")
    A8_d = nc.dram_tensor("A8", [GPC, NET, 4, 128, NPG], f8, kind="ExternalInput")
    We_d = nc.dram_tensor("We8", [2, 128, NET * HID], f8, kind="ExternalInput")
    wih_d = nc.dram_tensor("wih8", [2, 128, 3 * HID], f8, kind="ExternalInput")
    whh_d = nc.dram_tensor("whh8", [2, 128, 3 * HID], f8, kind="ExternalInput")
    rzb_d = nc.dram_tensor("rzb", [128, 4], f32, kind="ExternalInput")
    ginb_d = nc.dram_tensor("ginb", [128, 2], f32, kind="ExternalInput")
    ghnb_d = nc.dram_tensor("ghnb8", [128, 2], f32, kind="ExternalInput")
    beT_d = nc.dram_tensor("beT", [NET, HID], bf16, kind="ExternalInput")
    indeg_d = nc.dram_tensor("indeg", [GPC, NET, NPG], bf16, kind="ExternalInput")
    w1_d = nc.dram_tensor("w1D", [3, 128, 2, HID], f8, kind="ExternalInput")
    b1_d = nc.dram_tensor("b1", [128, 2], f32, kind="ExternalInput")
    w2_d = nc.dram_tensor("w2T", [2, 128, HID], bf16, kind="ExternalInput")
    b2_d = nc.dram_tensor("b2", [128, 2], f32, kind="ExternalInput")
    wc1_d = nc.dram_tensor("wc1D", [3, 128, 2, 2, 2 * HID], f8, kind="ExternalInput")
    bc1_d = nc.dram_tensor("bc1", [128, 4], f32, kind="ExternalInput")
    wc2_d = nc.dram_tensor("wc2T", [4, 128, 2 * HID], bf16, kind="ExternalInput")
    bc2_d = nc.dram_tensor("bc2", [128, 4], f32, kind="ExternalInput")
    wy_d = nc.dram_tensor("wy", [128, 2], bf16, kind="ExternalInput")
    wz_d = nc.dram_tensor("wz", [128, 4], bf16, kind="ExternalInput")
    byz_d = nc.dram_tensor("byz", [1, 2], f32, kind="ExternalInput")
    out_d = nc.dram_tensor("out", [1, GPC], f32, kind="ExternalOutput")
    if not conv:
        hdump_d = nc.dram_tensor("hdump", [128, 2, NLOC], bf16, kind="ExternalOutput")
        adump_d = nc.dram_tensor("adump", [128, 2, NLOC], f8, kind="ExternalOutput")

    # evac engine cycle: 'A' = Act, 'D' = DVE
    if evac_pat is None:
        evac_pat = "ADADADADADADA" if merge_evac else "ADADADADADADADADADADADADDD"
    if tn_bufs is None:
        tn_bufs = 2 if merge_evac else 4

    with tile.TileContext(nc) as tc, ExitStack() as top:
        state = top.enter_context(tc.tile_pool(name="state", bufs=1))
        h_t = state.tile([128, 2, NLOC], bf16)    # h master (bf16), hid-major
        h8_t = state.tile([128, 2, NLOC], f8)     # fp8 shadow of h for matmuls
        xD_t = state.tile([128, 2, NLOC], f8)     # fp8 x (conv head)
        adump_t = None
        if not conv:
            adump_t = state.tile([128, 2, NLOC], f8)

        cc = top.enter_context(tc.tile_pool(name="cc", bufs=1))
        w1_t = cc.tile([128, 3, 2, HID], f8)
        b1_t = cc.tile([128, 2], f32)
        w2_t = cc.tile([128, 2, HID], bf16)
        b2_t = cc.tile([128, 2], f32)
        wc1_t = cc.tile([128, 3, 2, 2, 2 * HID], f8)
        bc1_t = cc.tile([128, 4], f32)
        wc2_t = cc.tile([128, 4, 2 * HID], bf16)
        bc2_t = cc.tile([128, 4], f32)
        wy_t = cc.tile([128, 2], bf16)
        wz_t = cc.tile([128, 4], bf16)
        byz_t = cc.tile([1, 2], f32)

        # ---------------- GGNN: 6 message-passing + GRU steps ----------------
        with ExitStack() as gg:
            cg = gg.enter_context(tc.tile_pool(name="cg", bufs=1))
            We_t = cg.tile([128, 2, NET * HID], f8)
            # first kgroup's We so unit 0 can start early
            nc.sync.dma_start(
                We_t[:, :, 0:2 * HID],
                We_d[:, :, 0:2 * HID].rearrange("c p h -> p c h"),
            )
            # graph 0 x in half-graph chunks: first units only need nodes 0:256
            for half in range(2):
                hsl = slice(half * 256, (half + 1) * 256)
                nc.sync.dma_start(h8_t[:, :, hsl], xT8_d[:, :, hsl].rearrange("c p n -> p c n"))
                nc.sync.dma_start(h_t[:, :, hsl], xT_d[:, :, hsl].rearrange("c p n -> p c n"))
            nc.sync.dma_start(
                We_t[:, :, 2 * HID:],
                We_d[:, :, 2 * HID:].rearrange("c p h -> p c h"),
            )
            Ap = gg.enter_context(tc.tile_pool(name="Ap", bufs=1))
            A_t = Ap.tile([128, GPC, NET, 4, NPG], f8)
            for k in range(NET):
                nc.sync.dma_start(
                    A_t[:, 0, k, :, :], A8_d[0, k].rearrange("m p d -> p m d")
                )
            wih_t = cg.tile([128, 2, 3 * HID], f8)
            nc.sync.dma_start(wih_t[:], wih_d.rearrange("c p m -> p c m"))
            whh_t = cg.tile([128, 2, 3 * HID], f8)
            nc.sync.dma_start(whh_t[:], whh_d.rearrange("c p m -> p c m"))
            rzb_t = cg.tile([128, 4], f32)
            nc.sync.dma_start(rzb_t[:], rzb_d[:])
            ginb_t = cg.tile([128, 2], f32)
            nc.sync.dma_start(ginb_t[:], ginb_d[:])
            ghnb_t = cg.tile([128, 2], f32)
            nc.sync.dma_start(ghnb_t[:], ghnb_d[:])
            beT_t = cg.tile([NET, HID], bf16)
            nc.sync.dma_start(beT_t[:], beT_d[:])
            indeg_t = cg.tile([NET, GPC, NPG], bf16)
            nc.sync.dma_start(indeg_t[:], indeg_d.rearrange("g k d -> k g d"))
            # remaining graphs
            for g in range(1, GPC):
                gsl = slice(g * NPG, (g + 1) * NPG)
                nc.sync.dma_start(h8_t[:, :, gsl], xT8_d[:, :, gsl].rearrange("c p n -> p c n"))
                nc.sync.dma_start(h_t[:, :, gsl], xT_d[:, :, gsl].rearrange("c p n -> p c n"))
                for k in range(NET):
                    nc.sync.dma_start(
                        A_t[:, g, k, :, :], A8_d[g, k].rearrange("m p d -> p m d")
                    )
            nc.sync.dma_start(xD_t[:], xT8_d.rearrange("c p n -> p c n"))

            nc.sync.dma_start(w1_t[:], w1_d.rearrange("t p c o -> p t c o"))
            nc.sync.dma_start(b1_t[:], b1_d[:])
            nc.sync.dma_start(w2_t[:], w2_d.rearrange("c p o -> p c o"))
            nc.sync.dma_start(b2_t[:], b2_d[:])
            nc.sync.dma_start(wc1_t[:], wc1_d.rearrange("t p a b o -> p t a b o"))
            nc.sync.dma_start(bc1_t[:], bc1_d[:])
            nc.sync.dma_start(wc2_t[:], wc2_d.rearrange("c p o -> p c o"))
            nc.sync.dma_start(bc2_t[:], bc2_d[:])
            nc.sync.dma_start(wy_t[:], wy_d[:])
            nc.sync.dma_start(wz_t[:], wz_d[:])
            nc.sync.dma_start(byz_t[:], byz_d[:])

            ps_tn = gg.enter_context(tc.tile_pool(name="ps_tn", bufs=tn_bufs, space="PSUM"))
            ps_aT = gg.enter_context(tc.tile_pool(name="ps_aT", bufs=1, space="PSUM"))
            ps_gru = gg.enter_context(tc.tile_pool(name="ps_gru", bufs=2, space="PSUM"))
            tn_p = gg.enter_context(tc.tile_pool(name="tn", bufs=(skew + 1) if tnp_bufs is None else tnp_bufs))
            wk = gg.enter_context(tc.tile_pool(name="wk", bufs=wk_bufs))

            # ---- global software pipeline over all (step, graph) blocks ----
            SKEW = skew
            units = [(2 * q, 2, pi) for q in range(6) for pi in range(2)] + [(12, 1, 0)]
            NU = len(units)
            blocks = [(s, g) for s in range(steps) for g in range(GPC)]
            pend = []   # (tnD, s, g, k0, nk, pi, idx)
            aT_of = {}
            defq = []   # (due_tick, seq, closure)
            seqn = [0]
            tick = 0

            def defer(dt, fn):
                seqn[0] += 1
                defq.append((tick + dt, seqn[0], fn))

            def drain():
                defq.sort(key=lambda x: (x[0], x[1]))
                while defq and defq[0][0] <= tick:
                    defq.pop(0)[2]()

            evac_i = [0]

            def evac_engine():
                e = evac_pat[evac_i[0] % len(evac_pat)]
                evac_i[0] += 1
                return e

            def evac(dst_ap, src_ap):
                if evac_engine() == "A":
                    nc.scalar.activation(dst_ap, src_ap, AF.Identity, scale=0.125)
                else:
                    nc.vector.tensor_scalar_mul(dst_ap, src_ap, 0.125)

            def emit_s1(s, g, k0, nk, pi):
                # S1 PSUM: one 2-bank tile + a single ap-1024 evac (merge), or
                # two 1-bank tiles with two parallel ap-512 evacs. The fp8
                # SBUF tile is contiguous either way (the S2 DoubleRow
                # pair-dim spans both node-halves).
                if nk == 2:
                    tnD = tn_p.tile([128, 2, 2, HID], f8)
                    if merge_evac:
                        tp = ps_tn.tile([128, 2, 2, HID], f32, tag="tn_ps")
                        for j in range(2):
                            m = g * 4 + 2 * pi + j
                            msl = slice(m * 128, (m + 1) * 128)
                            nc.tensor.matmul(
                                tp[:, j, :, :], h8_t[:, :, msl],
                                We_t[:, :, k0 * HID:(k0 + nk) * HID],
                                start=True, stop=True, perf_mode=DR,
                            )
                        evac(tnD[:], tp[:])
                    else:
                        for j in range(2):
                            tp = ps_tn.tile([128, 2, HID], f32, tag="tn_ps")
                            m = g * 4 + 2 * pi + j
                            msl = slice(m * 128, (m + 1) * 128)
                            nc.tensor.matmul(
                                tp[:], h8_t[:, :, msl],
                                We_t[:, :, k0 * HID:(k0 + nk) * HID],
                                start=True, stop=True, perf_mode=DR,
                            )
                            evac(tnD[:, j, :, :], tp[:])
                else:
                    tnD = tn_p.tile([128, 4, HID], f8)
                    if merge_evac:
                        tp = ps_tn.tile([128, 4, HID], f32, tag="tn_ps")
                        for j in range(4):
                            m = g * 4 + j
                            msl = slice(m * 128, (m + 1) * 128)
                            nc.tensor.matmul(
                                tp[:, j, :], h8_t[:, :, msl],
                                We_t[:, :, k0 * HID:(k0 + 1) * HID],
                                start=True, stop=True, perf_mode=DR,
                            )
                        evac(tnD[:], tp[:])
                    else:
                        for q in range(2):
                            tp = ps_tn.tile([128, 2, HID], f32, tag="tn_ps")
                            for j in range(2):
                                m = g * 4 + 2 * q + j
                                msl = slice(m * 128, (m + 1) * 128)
                                nc.tensor.matmul(
                                    tp[:, j, :], h8_t[:, :, msl],
                                    We_t[:, :, k0 * HID:(k0 + 1) * HID],
                                    start=True, stop=True, perf_mode=DR,
                                )
                            evac(tnD[:, 2 * q:2 * q + 2, :], tp[:])
                return tnD

            def emit_s2(item):
                tnD, s, g, k0, nk, pi, idx = item
                if idx == 0:
                    aT_of[g] = ps_aT.tile([128, 2, NPG], f32, name="aT_ps", tag="aT_ps")
                    # seed the accumulator with the aggregated edge-bias:
                    # aT[hid, d] = sum_k b_e[k, hid] * indeg_k[d]
                    for hc in range(2):
                        nc.tensor.matmul(
                            aT_of[g][:, hc, :],
                            beT_t[:, hc * 128:(hc + 1) * 128],
                            indeg_t[:, g, :],
                            start=True, stop=False,
                        )
                aT_ps = aT_of[g]
                last = idx == NU - 1
                if nk == 2:
                    for hc in range(2):
                        for ko in range(2):
                            nc.tensor.matmul(
                                aT_ps[:, hc, :],
                                tnD[:, :, ko, hc * 128:(hc + 1) * 128],
                                A_t[:, g, k0 + ko, 2 * pi:2 * pi + 2, :],
                                start=False,
                                stop=(last and ko == 1),
                                perf_mode=DR,
                            )
                else:
                    for hc in range(2):
                        for qi in range(2):
                            nc.tensor.matmul(
                                aT_ps[:, hc, :],
                                tnD[:, 2 * qi:2 * qi + 2, hc * 128:(hc + 1) * 128],
                                A_t[:, g, k0, 2 * qi:2 * qi + 2, :],
                                start=False,
                                stop=(last and qi == 1),
                                perf_mode=DR,
                            )

            def emit_gru(s, g):
                gsl = slice(g * NPG, (g + 1) * NPG)
                aT_ps = aT_of.pop(g)
                aT8 = wk.tile([128, 2, NPG], f8, tag="aT8")
                r_t = wk.tile([128, 2, NPG], bf16, tag="r")
                z_t = wk.tile([128, 2, NPG], bf16, tag="z")
                n_t = wk.tile([128, 2, NPG], bf16, tag="n")
                d_t = r_t  # r is dead after n_gate's stt; reuse as ell scratch

                # now: evacuate aT (fp8 for the DR wih matmuls)
                for hc in range(2):
                    if at_halves[hc] == "A":
                        nc.scalar.copy(aT8[:, hc, :], aT_ps[:, hc, :])
                    else:
                        nc.vector.tensor_copy(aT8[:, hc, :], aT_ps[:, hc, :])
                if adump_t is not None and s == steps - 1:
                    nc.gpsimd.tensor_copy(adump_t[:, :, gsl], aT8[:])

                def gate_pair(jp):
                    # jp=0: r gates (jc 0,1); jp=1: z gates (jc 2,3)
                    def fn():
                        dst = r_t if jp == 0 else z_t
                        for hc in range(2):
                            jc = 2 * jp + hc
                            csl = slice(jc * 128, (jc + 1) * 128)
                            p_ = ps_gru.tile([128, NPG], f32, tag="gru")
                            nc.tensor.matmul(
                                p_[:], whh_t[:, :, csl], h8_t[:, :, gsl],
                                start=True, stop=False, perf_mode=DR,
                            )
                            nc.tensor.matmul(
                                p_[:], wih_t[:, :, csl], aT8[:],
                                start=False, stop=True, perf_mode=DR,
                            )
                            nc.scalar.activation(
                                dst[:, hc, :], p_[:], AF.Sigmoid,
                                bias=rzb_t[:, jc:jc + 1], scale=0.125,
                            )
                    return fn

                def n_gate(hc):
                    def fn():
                        csl = slice(512 + hc * 128, 512 + (hc + 1) * 128)
                        p_ = ps_gru.tile([128, NPG], f32, tag="gru")
                        nc.tensor.matmul(
                            p_[:], whh_t[:, :, csl], h8_t[:, :, gsl],
                            start=True, stop=True, perf_mode=DR,
                        )
                        # d = 8*r*(gh_n + bhh_n) to SBUF; gi_n in its own PSUM
                        # group; the bih_n bias is premixed (x8) so one merged
                        # tanh covers both hc
                        nc.vector.scalar_tensor_tensor(
                            d_t[:, hc, :], p_[:], ghnb_t[:, hc:hc + 1],
                            r_t[:, hc, :], op0=ALU.add, op1=ALU.mult,
                        )
                        p2 = ps_gru.tile([128, NPG], f32, tag="gru")
                        nc.tensor.matmul(
                            p2[:], wih_t[:, :, csl], aT8[:],
                            start=True, stop=True, perf_mode=DR,
                        )
                        nc.vector.scalar_tensor_tensor(
                            n_t[:, hc, :], p2[:], ginb_t[:, hc:hc + 1],
                            d_t[:, hc, :], op0=ALU.add, op1=ALU.add,
                        )
                        if hc == 1:
                            nc.scalar.activation(
                                n_t[:], n_t[:], AF.Tanh, scale=0.125,
                            )
                    return fn

                def ell():
                    # h' = n + z*(h - n); all-bf16 SBUF chain hits DVE's 4x
                    # mode. GpSimd does the fp8 shadow copy (TensorTensor
                    # arith fails the Pool ISA check on this toolchain).
                    for hc in range(2):
                        nc.vector.tensor_sub(d_t[:, hc, :], h_t[:, hc, gsl], n_t[:, hc, :])
                        nc.vector.tensor_mul(d_t[:, hc, :], d_t[:, hc, :], z_t[:, hc, :])
                        nc.vector.tensor_add(h_t[:, hc, gsl], n_t[:, hc, :], d_t[:, hc, :])
                        nc.gpsimd.tensor_copy(h8_t[:, hc, gsl], h_t[:, hc, gsl])

                defer(gdl[0], gate_pair(0))
                defer(gdl[1], gate_pair(1))
                defer(gdl[2], n_gate(0))
                defer(gdl[3], n_gate(1))
                defer(gdl[4], ell)

            for (s, g) in blocks:
                for idx, (k0, nk, pi) in enumerate(units):
                    tick += 1
                    tnD = emit_s1(s, g, k0, nk, pi)
                    pend.append((tnD, s, g, k0, nk, pi, idx))
                    if len(pend) > SKEW:
                        item = pend.pop(0)
                        emit_s2(item)
                        if item[6] == NU - 1:
                            emit_gru(item[1], item[2])
                    drain()
            while pend:
                tick += 1
                item = pend.pop(0)
                emit_s2(item)
                if item[6] == NU - 1:
                    emit_gru(item[1], item[2])
                drain()
            while defq:
                tick += 1
                drain()

        # ---------------- conv head + readout ----------------
        if conv:
            with ExitStack() as cv:
                L1, L2, L3 = 510, 254, 127
                outp = cv.enter_context(tc.tile_pool(name="outp", bufs=1))
                out_sb = outp.tile([1, GPC], f32)
                cw = cv.enter_context(tc.tile_pool(name="cw", bufs=2))
                psc = cv.enter_context(tc.tile_pool(name="psc", bufs=3, space="PSUM"))
                psr = cv.enter_context(tc.tile_pool(name="psr", bufs=1, space="PSUM"))

                # Phase A: K=3 convs + first maxpools for ALL graphs (keeps
                # cross-graph PE cover while the DVE pools run)
                y1s, z1s, y2s, z2s = {}, {}, {}, {}
                for g in range(GPC):
                    gof = g * NPG
                    y1p = cw.tile([128, 2, L1], bf16, tag="y1p")
                    for co in range(2):
                        p_ = psc.tile([128, L1], f32, tag="cps")
                        for t in range(3):
                            nc.tensor.matmul(
                                p_[:],
                                w1_t[:, t, :, co * 128:(co + 1) * 128],
                                h8_t[:, :, gof + t:gof + t + L1],
                                start=(t == 0), stop=(t == 2),
                                perf_mode=DR,
                            )
                        nc.scalar.activation(
                            y1p[:, co, :], p_[:], AF.Relu, bias=b1_t[:, co:co + 1]
                        )
                    y1 = cw.tile([128, 2, L2], bf16, tag="y1", bufs=4)
                    for co in range(2):
                        tp = cw.tile([128, L2], bf16, tag="tp", bufs=4)
                        e = y1p[:, co, :].rearrange("p (l s) -> p l s", s=2)
                        nc.vector.tensor_max(tp[:], e[:, :L2, 0], e[:, :L2, 1])
                        nc.vector.tensor_max(y1[:, co, :], tp[:], e[:, 1:L2 + 1, 0])
                    y1s[g] = y1

                    z1p = cw.tile([128, 4, L1], bf16, tag="z1p")
                    for co in range(4):
                        p_ = psc.tile([128, L1], f32, tag="cps")
                        idx = 0
                        for t in range(3):
                            for pr in range(2):
                                rhs = (h8_t if pr == 0 else xD_t)[
                                    :, :, gof + t:gof + t + L1
                                ]
                                nc.tensor.matmul(
                                    p_[:],
                                    wc1_t[:, t, pr, :, co * 128:(co + 1) * 128],
                                    rhs,
                                    start=(idx == 0), stop=(idx == 5),
                                    perf_mode=DR,
                                )
                                idx += 1
                        # relu on DVE: (psum + bias) max 0
                        nc.vector.tensor_scalar(
                            z1p[:, co, :], p_[:], bc1_t[:, co:co + 1], 0.0,
                            op0=ALU.add, op1=ALU.max,
                        )
                    z1 = cw.tile([128, 4, L2], bf16, tag="z1", bufs=4)
                    for co in range(4):
                        tp = cw.tile([128, L2], bf16, tag="tp2", bufs=4)
                        e = z1p[:, co, :].rearrange("p (l s) -> p l s", s=2)
                        nc.vector.tensor_max(tp[:], e[:, :L2, 0], e[:, :L2, 1])
                        nc.vector.tensor_max(z1[:, co, :], tp[:], e[:, 1:L2 + 1, 0])
                    z1s[g] = z1

                # Phase B: K=1 convs (bf16) + final pools
                for g in range(GPC):
                    y1, z1 = y1s[g], z1s[g]
                    y2p = cw.tile([128, 2, L2], bf16, tag="y2p")
                    for co in range(2):
                        p_ = psc.tile([128, L2], f32, tag="cps")
                        for ci in range(2):
                            nc.tensor.matmul(
                                p_[:], w2_t[:, ci, co * 128:(co + 1) * 128],
                                y1[:, ci, :],
                                start=(ci == 0), stop=(ci == 1),
                            )
                        nc.scalar.activation(
                            y2p[:, co, :], p_[:], AF.Relu, bias=b2_t[:, co:co + 1]
                        )
                    z2p = cw.tile([128, 4, L2], bf16, tag="z2p")
                    for co in range(4):
                        p_ = psc.tile([128, L2], f32, tag="cps")
                        for ci in range(4):
                            nc.tensor.matmul(
                                p_[:], wc2_t[:, ci, co * 128:(co + 1) * 128],
                                z1[:, ci, :],
                                start=(ci == 0), stop=(ci == 3),
                            )
                        nc.scalar.activation(
                            z2p[:, co, :], p_[:], AF.Relu, bias=bc2_t[:, co:co + 1]
                        )
                    y2 = cw.tile([128, 2, L3], bf16, tag="y2", bufs=4)
                    for co in range(2):
                        e = y2p[:, co, :].rearrange("p (l s) -> p l s", s=2)
                        nc.vector.tensor_max(y2[:, co, :], e[:, :, 0], e[:, :, 1])
                    z2 = cw.tile([128, 4, L3], bf16, tag="z2", bufs=4)
                    for co in range(4):
                        e = z2p[:, co, :].rearrange("p (l s) -> p l s", s=2)
                        nc.vector.tensor_max(z2[:, co, :], e[:, :, 0], e[:, :, 1])
                    y2s[g] = y2
                    z2s[g] = z2

                # Phase B2: readouts, covered by each other's matmuls
                for g in range(GPC):
                    y2, z2 = y2s[g], z2s[g]
                    zp = psr.tile([1, L3], f32, tag="rp")
                    for ci in range(4):
                        nc.tensor.matmul(
                            zp[:], wz_t[:, ci:ci + 1], z2[:, ci, :],
                            start=(ci == 0), stop=(ci == 3),
                        )
                    zb = cw.tile([1, L3], f32, tag="zb")
                    nc.vector.tensor_scalar_add(zb[:], zp[:], byz_t[:1, 1:2])
                    yp = psr.tile([1, L3], f32, tag="rp")
                    for hc in range(2):
                        nc.tensor.matmul(
                            yp[:], wy_t[:, hc:hc + 1], y2[:, hc, :],
                            start=(hc == 0), stop=(hc == 1),
                        )
                    yb = cw.tile([1, L3], f32, tag="yb")
                    sacc = cw.tile([1, 1], f32, tag="sacc")
                    nc.vector.scalar_tensor_tensor(
                        yb[:], yp[:], byz_t[:1, 0:1], zb[:],
                        op0=ALU.add, op1=ALU.mult,
                        accum_out=sacc[:],
                    )
                    nc.scalar.activation(
                        out_sb[:1, g:g + 1], sacc[:], AF.Sigmoid, scale=1.0 / L3
                    )
                nc.sync.dma_start(out_d[:], out_sb[:])
        else:
            with ExitStack() as cv:
                op = cv.enter_context(tc.tile_pool(name="outp", bufs=1))
                o_sb = op.tile([1, GPC], f32)
                nc.sync.dma_start(hdump_d[:], h_t[:])
                nc.sync.dma_start(adump_d[:], adump_t[:])
                nc.gpsimd.memset(o_sb[:], 0.0)
                nc.sync.dma_start(out_d[:], o_sb[:])

    nc.compile()
    return nc


def _host_prep(inputs):
    """Full inputs -> list of 8 per-core input dicts."""
    bf16 = ml_dtypes.bfloat16
    f8 = ml_dtypes.float8_e4m3

    x = np.asarray(inputs["x"], np.float32)
    src = np.asarray(inputs["src"], np.int32)
    dst = np.asarray(inputs["dst"], np.int32)
    et = np.asarray(inputs["etype"], np.int32)
    W_e = np.asarray(inputs["W_e"], np.float32)
    b_e = np.asarray(inputs["b_e"], np.float32)
    wih = np.asarray(inputs["gru_wih"], np.float32)
    whh = np.asarray(inputs["gru_whh"], np.float32)
    bih = np.asarray(inputs["gru_bih"], np.float32)
    bhh = np.asarray(inputs["gru_bhh"], np.float32)

    def wT8(w):  # [out, in] -> [2, 128, out] fp8, x8 (chunk-major dram layout)
        return np.ascontiguousarray(
            (8.0 * w).T.reshape(2, 128, w.shape[0])
        ).astype(f8)

    shared = {
        "We8": np.ascontiguousarray(
            (8.0 * W_e).reshape(NET, 2, 128, HID).transpose(1, 2, 0, 3)
            .reshape(2, 128, NET * HID)
        ).astype(f8),
        "wih8": wT8(wih),
        "whh8": wT8(whh),
        "rzb": np.ascontiguousarray((bih + bhh)[: 2 * HID].reshape(4, 128).T).astype(
            np.float32
        ),
        "ginb": np.ascontiguousarray(8.0 * bih[2 * HID:].reshape(2, 128).T).astype(
            np.float32
        ),
        "ghnb8": np.ascontiguousarray(8.0 * bhh[2 * HID:].reshape(2, 128).T).astype(
            np.float32
        ),
        "beT": np.ascontiguousarray(b_e).astype(bf16),
        "w1D": np.ascontiguousarray(
            np.transpose(
                np.transpose(np.asarray(inputs["conv1_w"], np.float32), (2, 1, 0))
                .reshape(3, 2, 128, HID), (0, 2, 1, 3)
            )
        ).astype(f8),
        "b1": np.ascontiguousarray(
            np.asarray(inputs["conv1_b"], np.float32).reshape(2, 128).T
        ),
        "w2T": np.ascontiguousarray(
            np.asarray(inputs["conv2_w"], np.float32)[:, :, 0].T.reshape(2, 128, HID)
        ).astype(bf16),
        "b2": np.ascontiguousarray(
            np.asarray(inputs["conv2_b"], np.float32).reshape(2, 128).T
        ),
        "wc1D": np.ascontiguousarray(
            np.transpose(
                np.transpose(np.asarray(inputs["cconv1_w"], np.float32), (2, 1, 0))
                .reshape(3, 2, 2, 128, 2 * HID), (0, 3, 1, 2, 4)
            )
        ).astype(f8),
        "bc1": np.ascontiguousarray(
            np.asarray(inputs["cconv1_b"], np.float32).reshape(4, 128).T
        ),
        "wc2T": np.ascontiguousarray(
            np.asarray(inputs["cconv2_w"], np.float32)[:, :, 0].T.reshape(
                4, 128, 2 * HID
            )
        ).astype(bf16),
        "bc2": np.ascontiguousarray(
            np.asarray(inputs["cconv2_b"], np.float32).reshape(4, 128).T
        ),
        "wy": np.ascontiguousarray(
            np.asarray(inputs["wy"], np.float32).reshape(2, 128).T
        ).astype(bf16),
        "wz": np.ascontiguousarray(
            np.asarray(inputs["wz"], np.float32).reshape(4, 128).T
        ).astype(bf16),
        "byz": np.array(
            [[float(np.asarray(inputs["by"]).reshape(-1)[0]),
              float(np.asarray(inputs["bz"]).reshape(-1)[0])]],
            np.float32,
        ),
    }

    in_maps = []
    for c in range(NCORES):
        n0 = c * NLOC
        esl = slice(c * GPC * EPG, (c + 1) * GPC * EPG)
        s_l = src[esl] - n0          # local node ids 0..2047
        d_l = dst[esl] - n0
        k_l = et[esl]
        g_l = s_l // NPG             # local graph 0..3 (edges stay in-graph)
        sg = s_l % NPG
        dg = d_l % NPG
        flat = ((g_l.astype(np.int64) * NET + k_l) * NPG + sg) * NPG + dg
        A = np.bincount(flat, minlength=GPC * NET * NPG * NPG).astype(f8)
        A8 = A.reshape(GPC, NET, 4, 128, NPG)

        dflat = (g_l.astype(np.int64) * NET + k_l) * NPG + dg
        indeg = np.bincount(dflat, minlength=GPC * NET * NPG).astype(bf16)

        xTc = np.ascontiguousarray(x[n0:n0 + NLOC].T.reshape(2, 128, NLOC))

        m = {
            "xT": xTc.astype(bf16),
            "xT8": xTc.astype(f8),
            "A8": A8,
            "indeg": indeg.reshape(GPC, NET, NPG),
        }
        m.update(shared)
        in_maps.append(m)
    return in_maps


def _get_nc():
    if "nc" not in _CACHE:
        _CACHE["nc"] = _build_nc()
    return _CACHE["nc"]


def run(inputs, trace=False):
    from concourse.bass_utils import run_bass_kernel_spmd

    nc = _get_nc()
    in_maps = _host_prep(inputs)
    res = run_bass_kernel_spmd(
        nc, in_maps, core_ids=list(range(NCORES)), trace=trace
    )
    out = np.concatenate(
        [np.asarray(res.results[c]["out"], np.float32).reshape(-1) for c in range(NCORES)]
    )
    return out, res


def kernel(**inputs):
    out, _ = run(inputs, trace=False)
    return out


# revision 76
# speedup vs baseline: 1.5902x; 1.0012x over previous
"""Devign-GGNN Trainium2 kernel.

Full inputs in, full output out. Sharding: data-parallel over the B=32
graphs -> 4 graphs per NeuronCore on 8 cores. The gather/scatter message
passing is reformulated as dense per-(graph, etype) adjacency matmuls:

    a = sum_k A_k^T (h @ W_k) + (indeg_k' b_k),  A_k[s, d] = #edges(s->d, k)

All three GGNN matmul families (h@W_e "stage 1", A^T "stage 2", GRU
gates) run as fp8e4m3 DoubleRow matmuls (contract 256/pass, 2 output
cols/cycle). W_e / GRU weights are scaled x8 host-side to sit in fp8's
normal range; the descale (x1/8) is folded into the PSUM-evacuation ops
(Act activation scale / DVE tensor_scalar) and the gate activations.

h master state is bf16 (DVE gets its 4x elementwise mode on all-bf16
SBUF ops) with an fp8 shadow h8 feeding the matmuls. The b_e
aggregation is a per-block rank-13 matmul (indeg x b_e^T) seeding the
stage-2 PSUM accumulator, replacing the host-side bincount. The conv
head keeps baseline numerics (fp8 K=3 convs, bf16 K=1 convs) - head
precision feeds the readout directly, so it gets the extra bits.

Engine split per GGNN block: PE ~10us of DR matmuls; Act (sigmoids,
tanh, ~half the tn PSUM->SBUF fp8 evacuations) and DVE (stt gates, GRU
state update, remaining evacuations) ~11us each; GpSimd only holds the
h->fp8 shadow copies (no PSUM port, and TensorTensor fails its ISA
check on this toolchain).
"""

import sys

if "/opt/trn_rl_repo" not in sys.path:
    sys.path.insert(0, "/opt/trn_rl_repo")

import numpy as np
import ml_dtypes

B, NPG, HID, NET, E, STEPS = 32, 512, 256, 13, 262144, 6
NCORES = 8
GPC = B // NCORES          # graphs per core = 4
NLOC = GPC * NPG           # local nodes = 2048
EPG = E // B               # edges per graph = 8192

_CACHE = {}


def _build_nc(steps=None, conv=None, skew=18, gdl=(3, 4, 6, 7, 10),
              evac_pat=None, wk_bufs=3, merge_evac=True, tn_bufs=None,
              at_halves="AA", tnp_bufs=14):
    steps = STEPS if steps is None else steps
    conv = True if conv is None else conv
    import concourse.bass as bass  # noqa: F401
    import concourse.tile as tile
    from concourse import mybir, bacc
    from contextlib import ExitStack

    f32 = mybir.dt.float32
    bf16 = mybir.dt.bfloat16
    f8 = mybir.dt.float8e4
    AF = mybir.ActivationFunctionType
    ALU = mybir.AluOpType
    DR = mybir.MatmulPerfMode.DoubleRow

    nc = bacc.Bacc(None, target_bir_lowering=False)

    xT_d = nc.dram_tensor("xT", [2, 128, NLOC], bf16, kind="ExternalInput")
    xT8_d = nc.dram_tensor("xT8", [2, 128, NLOC], f8, kind="ExternalInput")
    A8_d = nc.dram_tensor("A8", [GPC, NET, 4, 128, NPG], f8, kind="ExternalInput")
    We_d = nc.dram_tensor("We8", [2, 128, NET * HID], f8, kind="ExternalInput")
    wih_d = nc.dram_tensor("wih8", [2, 128, 3 * HID], f8, kind="ExternalInput")
    whh_d = nc.dram_tensor("whh8", [2, 128, 3 * HID], f8, kind="ExternalInput")
    rzb_d = nc.dram_tensor("rzb", [128, 4], f32, kind="ExternalInput")
    ginb_d = nc.dram_tensor("ginb", [128, 2], f32, kind="ExternalInput")
    ghnb_d = nc.dram_tensor("ghnb8", [128, 2], f32, kind="ExternalInput")
    beT_d = nc.dram_tensor("beT", [NET, HID], bf16, kind="ExternalInput")
    indeg_d = nc.dram_tensor("indeg", [GPC, NET, NPG], bf16, kind="ExternalInput")
    w1_d = nc.dram_tensor("w1D", [3, 128, 2, HID], f8, kind="ExternalInput")
    b1_d = nc.dram_tensor("b1", [128, 2], f32, kind="ExternalInput")
    w2_d = nc.dram_tensor("w2T", [2, 128, HID], bf16, kind="ExternalInput")
    b2_d = nc.dram_tensor("b2", [128, 2], f32, kind="ExternalInput")
    wc1_d = nc.dram_tensor("wc1D", [3, 128, 2, 2, 2 * HID], f8, kind="ExternalInput")
    bc1_d = nc.dram_tensor("bc1", [128, 4], f32, kind="ExternalInput")
    wc2_d = nc.dram_tensor("wc2T", [4, 128, 2 * HID], bf16, kind="ExternalInput")
    bc2_d = nc.dram_tensor("bc2", [128, 4], f32, kind="ExternalInput")
    wy_d = nc.dram_tensor("wy", [128, 2], bf16, kind="ExternalInput")
    wz_d = nc.dram_tensor("wz", [128, 4], bf16, kind="ExternalInput")
    byz_d = nc.dram_tensor("byz", [1, 2], f32, kind="ExternalInput")
    out_d = nc.dram_tensor("out", [1, GPC], f32, kind="ExternalOutput")
    if not conv:
        hdump_d = nc.dram_tensor("hdump", [128, 2, NLOC], bf16, kind="ExternalOutput")
        adump_d = nc.dram_tensor("adump", [128, 2, NLOC], f8, kind="ExternalOutput")

    # evac engine cycle: 'A' = Act, 'D' = DVE
    if evac_pat is None:
        evac_pat = "ADADADADADADAAD" if merge_evac else "ADADADADADADADADADADADADDD"
    if tn_bufs is None:
        tn_bufs = 2 if merge_evac else 4

    with tile.TileContext(nc) as tc, ExitStack() as top:
        state = top.enter_context(tc.tile_pool(name="state", bufs=1))
        h_t = state.tile([128, 2, NLOC], bf16)    # h master (bf16), hid-major
        h8_t = state.tile([128, 2, NLOC], f8)     # fp8 shadow of h for matmuls
        xD_t = state.tile([128, 2, NLOC], f8)     # fp8 x (conv head)
        adump_t = None
        if not conv:
            adump_t = state.tile([128, 2, NLOC], f8)

        cc = top.enter_context(tc.tile_pool(name="cc", bufs=1))
        w1_t = cc.tile([128, 3, 2, HID], f8)
        b1_t = cc.tile([128, 2], f32)
        w2_t = cc.tile([128, 2, HID], bf16)
        b2_t = cc.tile([128, 2], f32)
        wc1_t = cc.tile([128, 3, 2, 2, 2 * HID], f8)
        bc1_t = cc.tile([128, 4], f32)
        wc2_t = cc.tile([128, 4, 2 * HID], bf16)
        bc2_t = cc.tile([128, 4], f32)
        wy_t = cc.tile([128, 2], bf16)
        wz_t = cc.tile([128, 4], bf16)
        byz_t = cc.tile([1, 2], f32)

        # ---------------- GGNN: 6 message-passing + GRU steps ----------------
        with ExitStack() as gg:
            cg = gg.enter_context(tc.tile_pool(name="cg", bufs=1))
            We_t = cg.tile([128, 2, NET * HID], f8)
            # first kgroup's We so unit 0 can start early
            nc.sync.dma_start(
                We_t[:, :, 0:2 * HID],
                We_d[:, :, 0:2 * HID].rearrange("c p h -> p c h"),
            )
            # graph 0 x in half-graph chunks: first units only need nodes 0:256
            for half in range(2):
                hsl = slice(half * 256, (half + 1) * 256)
                nc.sync.dma_start(h8_t[:, :, hsl], xT8_d[:, :, hsl].rearrange("c p n -> p c n"))
                nc.sync.dma_start(h_t[:, :, hsl], xT_d[:, :, hsl].rearrange("c p n -> p c n"))
            nc.sync.dma_start(
                We_t[:, :, 2 * HID:],
                We_d[:, :, 2 * HID:].rearrange("c p h -> p c h"),
            )
            Ap = gg.enter_context(tc.tile_pool(name="Ap", bufs=1))
            A_t = Ap.tile([128, GPC, NET, 4, NPG], f8)
            for k in range(NET):
                nc.sync.dma_start(
                    A_t[:, 0, k, :, :], A8_d[0, k].rearrange("m p d -> p m d")
                )
            wih_t = cg.tile([128, 2, 3 * HID], f8)
            nc.sync.dma_start(wih_t[:], wih_d.rearrange("c p m -> p c m"))
            whh_t = cg.tile([128, 2, 3 * HID], f8)
            nc.sync.dma_start(whh_t[:], whh_d.rearrange("c p m -> p c m"))
            rzb_t = cg.tile([128, 4], f32)
            nc.sync.dma_start(rzb_t[:], rzb_d[:])
            ginb_t = cg.tile([128, 2], f32)
            nc.sync.dma_start(ginb_t[:], ginb_d[:])
            ghnb_t = cg.tile([128, 2], f32)
            nc.sync.dma_start(ghnb_t[:], ghnb_d[:])
            beT_t = cg.tile([NET, HID], bf16)
            nc.sync.dma_start(beT_t[:], beT_d[:])
            indeg_t = cg.tile([NET, GPC, NPG], bf16)
            nc.sync.dma_start(indeg_t[:], indeg_d.rearrange("g k d -> k g d"))
            # remaining graphs
            for g in range(1, GPC):
                gsl = slice(g * NPG, (g + 1) * NPG)
                nc.sync.dma_start(h8_t[:, :, gsl], xT8_d[:, :, gsl].rearrange("c p n -> p c n"))
                nc.sync.dma_start(h_t[:, :, gsl], xT_d[:, :, gsl].rearrange("c p n -> p c n"))
                for k in range(NET):
                    nc.sync.dma_start(
                        A_t[:, g, k, :, :], A8_d[g, k].rearrange("m p d -> p m d")
                    )
            nc.sync.dma_start(xD_t[:], xT8_d.rearrange("c p n -> p c n"))

            nc.sync.dma_start(w1_t[:], w1_d.rearrange("t p c o -> p t c o"))
            nc.sync.dma_start(b1_t[:], b1_d[:])
            nc.sync.dma_start(w2_t[:], w2_d.rearrange("c p o -> p c o"))
            nc.sync.dma_start(b2_t[:], b2_d[:])
            nc.sync.dma_start(wc1_t[:], wc1_d.rearrange("t p a b o -> p t a b o"))
            nc.sync.dma_start(bc1_t[:], bc1_d[:])
            nc.sync.dma_start(wc2_t[:], wc2_d.rearrange("c p o -> p c o"))
            nc.sync.dma_start(bc2_t[:], bc2_d[:])
            nc.sync.dma_start(wy_t[:], wy_d[:])
            nc.sync.dma_start(wz_t[:], wz_d[:])
            nc.sync.dma_start(byz_t[:], byz_d[:])

            ps_tn = gg.enter_context(tc.tile_pool(name="ps_tn", bufs=tn_bufs, space="PSUM"))
            ps_aT = gg.enter_context(tc.tile_pool(name="ps_aT", bufs=1, space="PSUM"))
            ps_gru = gg.enter_context(tc.tile_pool(name="ps_gru", bufs=2, space="PSUM"))
            tn_p = gg.enter_context(tc.tile_pool(name="tn", bufs=(skew + 1) if tnp_bufs is None else tnp_bufs))
            wk = gg.enter_context(tc.tile_pool(name="wk", bufs=wk_bufs))

            # ---- global software pipeline over all (step, graph) blocks ----
            SKEW = skew
            units = [(2 * q, 2, pi) for q in range(6) for pi in range(2)] + [(12, 1, 0)]
            NU = len(units)
            blocks = [(s, g) for s in range(steps) for g in range(GPC)]
            pend = []   # (tnD, s, g, k0, nk, pi, idx)
            aT_of = {}
            defq = []   # (due_tick, seq, closure)
            seqn = [0]
            tick = 0

            def defer(dt, fn):
                seqn[0] += 1
                defq.append((tick + dt, seqn[0], fn))

            def drain():
                defq.sort(key=lambda x: (x[0], x[1]))
                while defq and defq[0][0] <= tick:
                    defq.pop(0)[2]()

            evac_i = [0]

            def evac_engine():
                e = evac_pat[evac_i[0] % len(evac_pat)]
                evac_i[0] += 1
                return e

            def evac(dst_ap, src_ap):
                if evac_engine() == "A":
                    nc.scalar.activation(dst_ap, src_ap, AF.Identity, scale=0.125)
                else:
                    nc.vector.tensor_scalar_mul(dst_ap, src_ap, 0.125)

            def emit_s1(s, g, k0, nk, pi):
                # S1 PSUM: one 2-bank tile + a single ap-1024 evac (merge), or
                # two 1-bank tiles with two parallel ap-512 evacs. The fp8
                # SBUF tile is contiguous either way (the S2 DoubleRow
                # pair-dim spans both node-halves).
                if nk == 2:
                    tnD = tn_p.tile([128, 2, 2, HID], f8)
                    if merge_evac:
                        tp = ps_tn.tile([128, 2, 2, HID], f32, tag="tn_ps")
                        for j in range(2):
                            m = g * 4 + 2 * pi + j
                            msl = slice(m * 128, (m + 1) * 128)
                            nc.tensor.matmul(
                                tp[:, j, :, :], h8_t[:, :, msl],
                                We_t[:, :, k0 * HID:(k0 + nk) * HID],
                                start=True, stop=True, perf_mode=DR,
                            )
                        evac(tnD[:], tp[:])
                    else:
                        for j in range(2):
                            tp = ps_tn.tile([128, 2, HID], f32, tag="tn_ps")
                            m = g * 4 + 2 * pi + j
                            msl = slice(m * 128, (m + 1) * 128)
                            nc.tensor.matmul(
                                tp[:], h8_t[:, :, msl],
                                We_t[:, :, k0 * HID:(k0 + nk) * HID],
                                start=True, stop=True, perf_mode=DR,
                            )
                            evac(tnD[:, j, :, :], tp[:])
                else:
                    tnD = tn_p.tile([128, 4, HID], f8)
                    if merge_evac:
                        tp = ps_tn.tile([128, 4, HID], f32, tag="tn_ps")
                        for j in range(4):
                            m = g * 4 + j
                            msl = slice(m * 128, (m + 1) * 128)
                            nc.tensor.matmul(
                                tp[:, j, :], h8_t[:, :, msl],
                                We_t[:, :, k0 * HID:(k0 + 1) * HID],
                                start=True, stop=True, perf_mode=DR,
                            )
                        evac(tnD[:], tp[:])
                    else:
                        for q in range(2):
                            tp = ps_tn.tile([128, 2, HID], f32, tag="tn_ps")
                            for j in range(2):
                                m = g * 4 + 2 * q + j
                                msl = slice(m * 128, (m + 1) * 128)
                                nc.tensor.matmul(
                                    tp[:, j, :], h8_t[:, :, msl],
                                    We_t[:, :, k0 * HID:(k0 + 1) * HID],
                                    start=True, stop=True, perf_mode=DR,
                                )
                            evac(tnD[:, 2 * q:2 * q + 2, :], tp[:])
                return tnD

            def emit_s2(item):
                tnD, s, g, k0, nk, pi, idx = item
                if idx == 0:
                    aT_of[g] = ps_aT.tile([128, 2, NPG], f32, name="aT_ps", tag="aT_ps")
                    # seed the accumulator with the aggregated edge-bias:
                    # aT[hid, d] = sum_k b_e[k, hid] * indeg_k[d]
                    for hc in range(2):
                        nc.tensor.matmul(
                            aT_of[g][:, hc, :],
                            beT_t[:, hc * 128:(hc + 1) * 128],
                            indeg_t[:, g, :],
                            start=True, stop=False,
                        )
                aT_ps = aT_of[g]
                last = idx == NU - 1
                if nk == 2:
                    for hc in range(2):
                        for ko in range(2):
                            nc.tensor.matmul(
                                aT_ps[:, hc, :],
                                tnD[:, :, ko, hc * 128:(hc + 1) * 128],
                                A_t[:, g, k0 + ko, 2 * pi:2 * pi + 2, :],
                                start=False,
                                stop=(last and ko == 1),
                                perf_mode=DR,
                            )
                else:
                    for hc in range(2):
                        for qi in range(2):
                            nc.tensor.matmul(
                                aT_ps[:, hc, :],
                                tnD[:, 2 * qi:2 * qi + 2, hc * 128:(hc + 1) * 128],
                                A_t[:, g, k0, 2 * qi:2 * qi + 2, :],
                                start=False,
                                stop=(last and qi == 1),
                                perf_mode=DR,
                            )

            def emit_gru(s, g):
                gsl = slice(g * NPG, (g + 1) * NPG)
                aT_ps = aT_of.pop(g)
                aT8 = wk.tile([128, 2, NPG], f8, tag="aT8")
                r_t = wk.tile([128, 2, NPG], bf16, tag="r")
                z_t = wk.tile([128, 2, NPG], bf16, tag="z")
                n_t = wk.tile([128, 2, NPG], bf16, tag="n")
                d_t = r_t  # r is dead after n_gate's stt; reuse as ell scratch

                # now: evacuate aT (fp8 for the DR wih matmuls)
                for hc in range(2):
                    if at_halves[hc] == "A":
                        nc.scalar.copy(aT8[:, hc, :], aT_ps[:, hc, :])
                    else:
                        nc.vector.tensor_copy(aT8[:, hc, :], aT_ps[:, hc, :])
                if adump_t is not None and s == steps - 1:
                    nc.gpsimd.tensor_copy(adump_t[:, :, gsl], aT8[:])

                def gate_pair(jp):
                    # jp=0: r gates (jc 0,1); jp=1: z gates (jc 2,3)
                    def fn():
                        dst = r_t if jp == 0 else z_t
                        for hc in range(2):
                            jc = 2 * jp + hc
                            csl = slice(jc * 128, (jc + 1) * 128)
                            p_ = ps_gru.tile([128, NPG], f32, tag="gru")
                            nc.tensor.matmul(
                                p_[:], whh_t[:, :, csl], h8_t[:, :, gsl],
                                start=True, stop=False, perf_mode=DR,
                            )
                            nc.tensor.matmul(
                                p_[:], wih_t[:, :, csl], aT8[:],
                                start=False, stop=True, perf_mode=DR,
                            )
                            nc.scalar.activation(
                                dst[:, hc, :], p_[:], AF.Sigmoid,
                                bias=rzb_t[:, jc:jc + 1], scale=0.125,
                            )
                    return fn

                def n_gate(hc):
                    def fn():
                        csl = slice(512 + hc * 128, 512 + (hc + 1) * 128)
                        p_ = ps_gru.tile([128, NPG], f32, tag="gru")
                        nc.tensor.matmul(
                            p_[:], whh_t[:, :, csl], h8_t[:, :, gsl],
                            start=True, stop=True, perf_mode=DR,
                        )
                        # d = 8*r*(gh_n + bhh_n) to SBUF; gi_n in its own PSUM
                        # group; the bih_n bias is premixed (x8) so one merged
                        # tanh covers both hc
                        nc.vector.scalar_tensor_tensor(
                            d_t[:, hc, :], p_[:], ghnb_t[:, hc:hc + 1],
                            r_t[:, hc, :], op0=ALU.add, op1=ALU.mult,
                        )
                        p2 = ps_gru.tile([128, NPG], f32, tag="gru")
                        nc.tensor.matmul(
                            p2[:], wih_t[:, :, csl], aT8[:],
                            start=True, stop=True, perf_mode=DR,
                        )
                        nc.vector.scalar_tensor_tensor(
                            n_t[:, hc, :], p2[:], ginb_t[:, hc:hc + 1],
                            d_t[:, hc, :], op0=ALU.add, op1=ALU.add,
                        )
                        if hc == 1:
                            nc.scalar.activation(
                                n_t[:], n_t[:], AF.Tanh, scale=0.125,
                            )
                    return fn

                def ell():
                    # h' = n + z*(h - n); all-bf16 SBUF chain hits DVE's 4x
                    # mode. GpSimd does the fp8 shadow copy (TensorTensor
                    # arith fails the Pool ISA check on this toolchain).
                    for hc in range(2):
                        nc.vector.tensor_sub(d_t[:, hc, :], h_t[:, hc, gsl], n_t[:, hc, :])
                        nc.vector.tensor_mul(d_t[:, hc, :], d_t[:, hc, :], z_t[:, hc, :])
                        nc.vector.tensor_add(h_t[:, hc, gsl], n_t[:, hc, :], d_t[:, hc, :])
                        nc.gpsimd.tensor_copy(h8_t[:, hc, gsl], h_t[:, hc, gsl])

                defer(gdl[0], gate_pair(0))
                defer(gdl[1], gate_pair(1))
                defer(gdl[2], n_gate(0))
                defer(gdl[3], n_gate(1))
                defer(gdl[4], ell)

            for (s, g) in blocks:
                for idx, (k0, nk, pi) in enumerate(units):
                    tick += 1
                    tnD = emit_s1(s, g, k0, nk, pi)
                    pend.append((tnD, s, g, k0, nk, pi, idx))
                    if len(pend) > SKEW:
                        item = pend.pop(0)
                        emit_s2(item)
                        if item[6] == NU - 1:
                            emit_gru(item[1], item[2])
                    drain()
            while pend:
                tick += 1
                item = pend.pop(0)
                emit_s2(item)
                if item[6] == NU - 1:
                    emit_gru(item[1], item[2])
                drain()
            while defq:
                tick += 1
                drain()

        # ---------------- conv head + readout ----------------
        if conv:
            with ExitStack() as cv:
                L1, L2, L3 = 510, 254, 127
                outp = cv.enter_context(tc.tile_pool(name="outp", bufs=1))
                out_sb = outp.tile([1, GPC], f32)
                cw = cv.enter_context(tc.tile_pool(name="cw", bufs=2))
                psc = cv.enter_context(tc.tile_pool(name="psc", bufs=3, space="PSUM"))
                psr = cv.enter_context(tc.tile_pool(name="psr", bufs=1, space="PSUM"))

                # Phase A: K=3 convs + first maxpools for ALL graphs (keeps
                # cross-graph PE cover while the DVE pools run)
                y1s, z1s, y2s, z2s = {}, {}, {}, {}
                for g in range(GPC):
                    gof = g * NPG
                    y1p = cw.tile([128, 2, L1], bf16, tag="y1p")
                    for co in range(2):
                        p_ = psc.tile([128, L1], f32, tag="cps")
                        for t in range(3):
                            nc.tensor.matmul(
                                p_[:],
                                w1_t[:, t, :, co * 128:(co + 1) * 128],
                                h8_t[:, :, gof + t:gof + t + L1],
                                start=(t == 0), stop=(t == 2),
                                perf_mode=DR,
                            )
                        nc.scalar.activation(
                            y1p[:, co, :], p_[:], AF.Relu, bias=b1_t[:, co:co + 1]
                        )
                    y1 = cw.tile([128, 2, L2], bf16, tag="y1", bufs=4)
                    for co in range(2):
                        tp = cw.tile([128, L2], bf16, tag="tp", bufs=4)
                        e = y1p[:, co, :].rearrange("p (l s) -> p l s", s=2)
                        nc.vector.tensor_max(tp[:], e[:, :L2, 0], e[:, :L2, 1])
                        nc.vector.tensor_max(y1[:, co, :], tp[:], e[:, 1:L2 + 1, 0])
                    y1s[g] = y1

                    z1p = cw.tile([128, 4, L1], bf16, tag="z1p")
                    for co in range(4):
                        p_ = psc.tile([128, L1], f32, tag="cps")
                        idx = 0
                        for t in range(3):
                            for pr in range(2):
                                rhs = (h8_t if pr == 0 else xD_t)[
                                    :, :, gof + t:gof + t + L1
                                ]
                                nc.tensor.matmul(
                                    p_[:],
                                    wc1_t[:, t, pr, :, co * 128:(co + 1) * 128],
                                    rhs,
                                    start=(idx == 0), stop=(idx == 5),
                                    perf_mode=DR,
                                )
                                idx += 1
                        # relu on DVE: (psum + bias) max 0
                        nc.vector.tensor_scalar(
                            z1p[:, co, :], p_[:], bc1_t[:, co:co + 1], 0.0,
                            op0=ALU.add, op1=ALU.max,
                        )
                    z1 = cw.tile([128, 4, L2], bf16, tag="z1", bufs=4)
                    for co in range(4):
                        tp = cw.tile([128, L2], bf16, tag="tp2", bufs=4)
                        e = z1p[:, co, :].rearrange("p (l s) -> p l s", s=2)
                        nc.vector.tensor_max(tp[:], e[:, :L2, 0], e[:, :L2, 1])
                        nc.vector.tensor_max(z1[:, co, :], tp[:], e[:, 1:L2 + 1, 0])
                    z1s[g] = z1

                # Phase B: K=1 convs (bf16) + final pools
                for g in range(GPC):
                    y1, z1 = y1s[g], z1s[g]
                    y2p = cw.tile([128, 2, L2], bf16, tag="y2p")
                    for co in range(2):
                        p_ = psc.tile([128, L2], f32, tag="cps")
                        for ci in range(2):
                            nc.tensor.matmul(
                                p_[:], w2_t[:, ci, co * 128:(co + 1) * 128],
                                y1[:, ci, :],
                                start=(ci == 0), stop=(ci == 1),
                            )
                        nc.scalar.activation(
                            y2p[:, co, :], p_[:], AF.Relu, bias=b2_t[:, co:co + 1]
                        )
                    z2p = cw.tile([128, 4, L2], bf16, tag="z2p")
                    for co in range(4):
                        p_ = psc.tile([128, L2], f32, tag="cps")
                        for ci in range(4):
                            nc.tensor.matmul(
                                p_[:], wc2_t[:, ci, co * 128:(co + 1) * 128],
                                z1[:, ci, :],
                                start=(ci == 0), stop=(ci == 3),
                            )
                        nc.scalar.activation(
                            z2p[:, co, :], p_[:], AF.Relu, bias=bc2_t[:, co:co + 1]
                        )
                    y2 = cw.tile([128, 2, L3], bf16, tag="y2", bufs=4)
                    for co in range(2):
                        e = y2p[:, co, :].rearrange("p (l s) -> p l s", s=2)
                        nc.vector.tensor_max(y2[:, co, :], e[:, :, 0], e[:, :, 1])
                    z2 = cw.tile([128, 4, L3], bf16, tag="z2", bufs=4)
                    for co in range(4):
                        e = z2p[:, co, :].rearrange("p (l s) -> p l s", s=2)
                        nc.vector.tensor_max(z2[:, co, :], e[:, :, 0], e[:, :, 1])
                    y2s[g] = y2
                    z2s[g] = z2

                # Phase B2: readouts, covered by each other's matmuls
                for g in range(GPC):
                    y2, z2 = y2s[g], z2s[g]
                    zp = psr.tile([1, L3], f32, tag="rp")
                    for ci in range(4):
                        nc.tensor.matmul(
                            zp[:], wz_t[:, ci:ci + 1], z2[:, ci, :],
                            start=(ci == 0), stop=(ci == 3),
                        )
                    zb = cw.tile([1, L3], f32, tag="zb")
                    nc.vector.tensor_scalar_add(zb[:], zp[:], byz_t[:1, 1:2])
                    yp = psr.tile([1, L3], f32, tag="rp")
                    for hc in range(2):
                        nc.tensor.matmul(
                            yp[:], wy_t[:, hc:hc + 1], y2[:, hc, :],
                            start=(hc == 0), stop=(hc == 1),
                        )
                    yb = cw.tile([1, L3], f32, tag="yb")
                    sacc = cw.tile([1, 1], f32, tag="sacc")
                    nc.vector.scalar_tensor_tensor(
                        yb[:], yp[:], byz_t[:1, 0:1], zb[:],
                        op0=ALU.add, op1=ALU.mult,
                        accum_out=sacc[:],
                    )
                    nc.scalar.activation(
                        out_sb[:1, g:g + 1], sacc[:], AF.Sigmoid, scale=1.0 / L3
                    )
                nc.sync.dma_start(out_d[:], out_sb[:])
        else:
            with ExitStack() as cv:
                op = cv.enter_context(tc.tile_pool(name="outp", bufs=1))
                o_sb = op.tile([1, GPC], f32)
                nc.sync.dma_start(hdump_d[:], h_t[:])
                nc.sync.dma_start(adump_d[:], adump_t[:])
                nc.gpsimd.memset(o_sb[:], 0.0)
                nc.sync.dma_start(out_d[:], o_sb[:])

    nc.compile()
    return nc


def _host_prep(inputs):
    """Full inputs -> list of 8 per-core input dicts."""
    bf16 = ml_dtypes.bfloat16
    f8 = ml_dtypes.float8_e4m3

    x = np.asarray(inputs["x"], np.float32)
    src = np.asarray(inputs["src"], np.int32)
    dst = np.asarray(inputs["dst"], np.int32)
    et = np.asarray(inputs["etype"], np.int32)
    W_e = np.asarray(inputs["W_e"], np.float32)
    b_e = np.asarray(inputs["b_e"], np.float32)
    wih = np.asarray(inputs["gru_wih"], np.float32)
    whh = np.asarray(inputs["gru_whh"], np.float32)
    bih = np.asarray(inputs["gru_bih"], np.float32)
    bhh = np.asarray(inputs["gru_bhh"], np.float32)

    def wT8(w):  # [out, in] -> [2, 128, out] fp8, x8 (chunk-major dram layout)
        return np.ascontiguousarray(
            (8.0 * w).T.reshape(2, 128, w.shape[0])
        ).astype(f8)

    shared = {
        "We8": np.ascontiguousarray(
            (8.0 * W_e).reshape(NET, 2, 128, HID).transpose(1, 2, 0, 3)
            .reshape(2, 128, NET * HID)
        ).astype(f8),
        "wih8": wT8(wih),
        "whh8": wT8(whh),
        "rzb": np.ascontiguousarray((bih + bhh)[: 2 * HID].reshape(4, 128).T).astype(
            np.float32
        ),
        "ginb": np.ascontiguousarray(8.0 * bih[2 * HID:].reshape(2, 128).T).astype(
            np.float32
        ),
        "ghnb8": np.ascontiguousarray(8.0 * bhh[2 * HID:].reshape(2, 128).T).astype(
            np.float32
        ),
        "beT": np.ascontiguousarray(b_e).astype(bf16),
        "w1D": np.ascontiguousarray(
            np.transpose(
                np.transpose(np.asarray(inputs["conv1_w"], np.float32), (2, 1, 0))
                .reshape(3, 2, 128, HID), (0, 2, 1, 3)
            )
        ).astype(f8),
        "b1": np.ascontiguousarray(
            np.asarray(inputs["conv1_b"], np.float32).reshape(2, 128).T
        ),
        "w2T": np.ascontiguousarray(
            np.asarray(inputs["conv2_w"], np.float32)[:, :, 0].T.reshape(2, 128, HID)
        ).astype(bf16),
        "b2": np.ascontiguousarray(
            np.asarray(inputs["conv2_b"], np.float32).reshape(2, 128).T
        ),
        "wc1D": np.ascontiguousarray(
            np.transpose(
                np.transpose(np.asarray(inputs["cconv1_w"], np.float32), (2, 1, 0))
                .reshape(3, 2, 2, 128, 2 * HID), (0, 3, 1, 2, 4)
            )
        ).astype(f8),
        "bc1": np.ascontiguousarray(
            np.asarray(inputs["cconv1_b"], np.float32).reshape(4, 128).T
        ),
        "wc2T": np.ascontiguousarray(
            np.asarray(inputs["cconv2_w"], np.float32)[:, :, 0].T.reshape(
                4, 128, 2 * HID
            )
        ).astype(bf16),
        "bc2": np.ascontiguousarray(
            np.asarray(inputs["cconv2_b"], np.float32).reshape(4, 128).T
        ),
        "wy": np.ascontiguousarray(
            np.asarray(inputs["wy"], np.float32).reshape(2, 128).T
        ).astype(bf16),
        "wz": np.ascontiguousarray(
            np.asarray(inputs["wz"], np.float32).reshape(4, 128).T
        ).astype(bf16),
        "byz": np.array(
            [[float(np.asarray(inputs["by"]).reshape(-1)[0]),
              float(np.asarray(inputs["bz"]).reshape(-1)[0])]],
            np.float32,
        ),
    }

    in_maps = []
    for c in range(NCORES):
        n0 = c * NLOC
        esl = slice(c * GPC * EPG, (c + 1) * GPC * EPG)
        s_l = src[esl] - n0          # local node ids 0..2047
        d_l = dst[esl] - n0
        k_l = et[esl]
        g_l = s_l // NPG             # local graph 0..3 (edges stay in-graph)
        sg = s_l % NPG
        dg = d_l % NPG
        flat = ((g_l.astype(np.int64) * NET + k_l) * NPG + sg) * NPG + dg
        A = np.bincount(flat, minlength=GPC * NET * NPG * NPG).astype(f8)
        A8 = A.reshape(GPC, NET, 4, 128, NPG)

        dflat = (g_l.astype(np.int64) * NET + k_l) * NPG + dg
        indeg = np.bincount(dflat, minlength=GPC * NET * NPG).astype(bf16)

        xTc = np.ascontiguousarray(x[n0:n0 + NLOC].T.reshape(2, 128, NLOC))

        m = {
            "xT": xTc.astype(bf16),
            "xT8": xTc.astype(f8),
            "A8": A8,
            "indeg": indeg.reshape(GPC, NET, NPG),
        }
        m.update(shared)
        in_maps.append(m)
    return in_maps


def _get_nc():
    if "nc" not in _CACHE:
        _CACHE["nc"] = _build_nc()
    return _CACHE["nc"]


def run(inputs, trace=False):
    from concourse.bass_utils import run_bass_kernel_spmd

    nc = _get_nc()
    in_maps = _host_prep(inputs)
    res = run_bass_kernel_spmd(
        nc, in_maps, core_ids=list(range(NCORES)), trace=trace
    )
    out = np.concatenate(
        [np.asarray(res.results[c]["out"], np.float32).reshape(-1) for c in range(NCORES)]
    )
    return out, res


def kernel(**inputs):
    out, _ = run(inputs, trace=False)
    return out
